# revision 2
# baseline (speedup 1.0000x reference)
"""Trainium2 Bass kernel for the FAWMF loss (gnn_message_passing).

Strategy (8 NeuronCores, SPMD, no collectives):
  - Only item-rows of z matter, and only for items referenced by the batch
    (all_theta[col]=0 for col>=U, z[:U] is discarded by the reference).
  - Each core takes 1/8 of the (users, pos, neg) batch plus exactly the edges
    feeding its batch's items (need-based edge sharding). Items are remapped to
    <=4096 local slots (32 blocks of 128), balanced by degree.
  - Device: softmax(theta_user) is computed on-chip and written to a padded
    [U, 64] DRAM table (256B rows). Edge cols are gathered from it with
    GPSIMD dma_gather (int16 indices, 4 range-buckets), scaled by edge vals,
    and segment-summed into z blocks via one-hot matmuls accumulating in PSUM.
    z1 = sigmoid(w1*z + w2) via ACT with per-partition scale/bias, written to
    a padded DRAM table; batch rows of z1/theta/embeddings are gathered back
    and reduced to 7 partial sums per core.
  - Host combines the 8 partial-sum vectors into the scalar loss.
"""
import os
import sys
import numpy as np

for _p in ("/opt/trn_rl_repo", "/root/.axon_site", "/root/.axon_site/_ro/trn_rl_repo",
           "/root/.axon_site/_ro/pypackages"):
    if os.path.isdir(_p) and _p not in sys.path:
        sys.path.append(_p)

import concourse.bacc as bacc
import concourse.bass as bass
import concourse.mybir as mybir
import concourse.tile as tile

f32 = mybir.dt.float32
i16 = mybir.dt.int16
AF = mybir.ActivationFunctionType
OP = mybir.AluOpType
AX = mybir.AxisListType

U, I, C, D, B = 100000, 50000, 32, 64, 16384
NCORES = 8
BPC = B // NCORES
NL = BPC // 128             # batch lanes per partition (16)
NBLK = 32                   # 128-row z blocks per core
NUB = 4                     # user-id buckets (int16 gather)
UBSZ = 25000
NIB = 2                     # item-id buckets
IBSZ = 32768
EPAD = 64                   # padded table row width (256B)
GRP = 4                     # blocks per gather group
NGRP = NBLK // GRP
NTT = 15                    # full softmax tiles (128 part x 50 rows)
QSM = 50                    # user rows per partition per softmax tile
WD = 1e-4

# ----------------------------------------------------------------------------
# host-side sharding helpers
# ----------------------------------------------------------------------------

def _wrap16(idx):
    n = idx.shape[0]
    a = idx.reshape(n // 16, 16).T.astype(np.int16)
    return np.ascontiguousarray(np.tile(a, (8, 1)))


def _lanes(a, ncol):
    return np.ascontiguousarray(a.reshape(ncol, 128).T)


def _prep(inputs):
    users = np.asarray(inputs["users"]).astype(np.int64)
    pos = np.asarray(inputs["positive_items"]).astype(np.int64)
    neg = np.asarray(inputs["negative_items"]).astype(np.int64)
    er_full = np.asarray(inputs["edge_rows"])
    ec_full = np.asarray(inputs["edge_cols"])
    ev_full = np.asarray(inputs["edge_vals"])
    w1 = np.asarray(inputs["w1"], np.float32).reshape(-1)
    w2 = np.asarray(inputs["w2"], np.float32).reshape(-1)
    theta = np.asarray(inputs["theta_user"], np.float32)
    uemb = np.asarray(inputs["user_embedding"], np.float32)
    iemb = np.asarray(inputs["item_embedding"], np.float32)

    m = (er_full >= U) & (ec_full < U)
    er = (er_full[m] - U).astype(np.int64)
    ec = ec_full[m].astype(np.int64)
    ev = np.asarray(ev_full[m], np.float32)
    order = np.argsort(er, kind="stable")
    er, ec, ev = er[order], ec[order], ev[order]
    counts = np.bincount(er, minlength=I)
    ptr = np.zeros(I + 1, np.int64)
    np.cumsum(counts, out=ptr[1:])

    import heapq
    cores = []
    S_needed = 1
    for k in range(NCORES):
        u_k = users[k * BPC:(k + 1) * BPC]
        p_k = pos[k * BPC:(k + 1) * BPC]
        n_k = neg[k * BPC:(k + 1) * BPC]
        ids = np.unique(np.concatenate([p_k, n_k]))
        deg = counts[ids]
        o = np.argsort(-deg, kind="stable")
        blk_of = np.empty(len(ids), np.int64)
        slot_in_blk = np.empty(len(ids), np.int64)
        heap = [(0, 0, b) for b in range(NBLK)]
        heapq.heapify(heap)
        for i_ in o:
            while True:
                load, cnt, b = heapq.heappop(heap)
                if cnt < 128:
                    break
            blk_of[i_] = b
            slot_in_blk[i_] = cnt
            heapq.heappush(heap, (load + int(deg[i_]), cnt + 1, b))
        slot = blk_of * 128 + slot_in_blk
        slot_of_item = np.full(I, -1, np.int64)
        slot_of_item[ids] = slot

        lens = counts[ids]
        tot = int(lens.sum())
        startp = ptr[ids]
        idx = np.repeat(startp + lens - np.cumsum(lens), lens) + np.arange(tot)
        e_col = ec[idx]
        e_val = ev[idx]
        e_slot = np.repeat(slot, lens)
        e_lo = (e_slot & 127).astype(np.float32)
        e_bkt = e_col // UBSZ
        seg_key = (e_slot >> 7) * NUB + e_bkt
        seg_cnt = np.bincount(seg_key, minlength=NBLK * NUB)
        S_needed = max(S_needed, int(np.ceil(seg_cnt.max() / 128)))
        cores.append(dict(u_k=u_k, p_k=p_k, n_k=n_k, ids=ids, slot=slot,
                          slot_of_item=slot_of_item, e_col=e_col, e_val=e_val,
                          e_lo=e_lo, seg_key=seg_key))

    S = S_needed
    SEGE = S * 128
    TOTCH = NBLK * NUB * S
    TOTE = TOTCH * 128
    NI = GRP * SEGE

    seg_order = [(g * GRP + b_loc, r)
                 for g in range(NGRP) for r in range(NUB) for b_loc in range(GRP)]

    in_maps = []
    for k in range(NCORES):
        c = cores[k]
        segs_cols = np.zeros(TOTE, np.int64)
        segs_vals = np.zeros(TOTE, np.float32)
        segs_lo = np.zeros(TOTE, np.float32)
        sort_by_seg = np.argsort(c["seg_key"], kind="stable")
        seg_sorted_cols = c["e_col"][sort_by_seg]
        seg_sorted_vals = c["e_val"][sort_by_seg]
        seg_sorted_lo = c["e_lo"][sort_by_seg]
        seg_cnt = np.bincount(c["seg_key"], minlength=NBLK * NUB)
        seg_ptr = np.zeros(NBLK * NUB + 1, np.int64)
        np.cumsum(seg_cnt, out=seg_ptr[1:])
        off = 0
        for (b, r) in seg_order:
            sk = b * NUB + r
            ns = int(seg_cnt[sk])
            sl = slice(seg_ptr[sk], seg_ptr[sk] + ns)
            segs_cols[off:off + ns] = seg_sorted_cols[sl]
            segs_vals[off:off + ns] = seg_sorted_vals[sl]
            segs_lo[off:off + ns] = seg_sorted_lo[sl]
            segs_cols[off + ns:off + SEGE] = r * UBSZ
            off += SEGE

        cols16_parts = []
        for gi in range(NGRP * NUB):
            r = gi % NUB
            rel = segs_cols[gi * NI:(gi + 1) * NI] - r * UBSZ
            cols16_parts.append(_wrap16(rel.astype(np.int16)))
        cols16 = np.concatenate(cols16_parts, axis=1)

        lo_t = _lanes(segs_lo, TOTCH)
        val_t = _lanes(segs_vals, TOTCH)

        som = c["slot_of_item"]
        p_slot = som[c["p_k"]]
        n_slot = som[c["n_k"]]

        def bucketize(idx_global, nb, bsz):
            arrs, masks = [], []
            for r in range(nb):
                inb = (idx_global >= r * bsz) & (idx_global < (r + 1) * bsz)
                rel = np.where(inb, idx_global - r * bsz, 0)
                arrs.append(_wrap16(rel.astype(np.int16)))
                masks.append(_lanes(inb.astype(np.float32), NL))
            return arrs, masks

        uidx, umask = bucketize(c["u_k"], NUB, UBSZ)
        pidx, pmask = bucketize(c["p_k"], NIB, IBSZ)
        nidx, nmask = bucketize(c["n_k"], NIB, IBSZ)

        w1_loc = np.zeros((128, NBLK), np.float32)
        w2_loc = np.zeros((128, NBLK), np.float32)
        slot = c["slot"]
        w1_loc[slot & 127, slot >> 7] = w1[c["ids"]]
        w2_loc[slot & 127, slot >> 7] = w2[c["ids"]]

        th_sl = np.ascontiguousarray(
            theta[k * (U // NCORES):(k + 1) * (U // NCORES)].reshape(128, -1))
        wcat = np.concatenate([w1[k * (I // NCORES):(k + 1) * (I // NCORES)],
                               w2[k * (I // NCORES):(k + 1) * (I // NCORES)]])
        wpad = np.zeros(128 * 98, np.float32)
        wpad[:len(wcat)] = wcat
        w_sq = np.ascontiguousarray(wpad.reshape(98, 128).T)

        in_map = {
            "theta": theta, "uemb": uemb, "iemb": iemb,
            "cols16": cols16.astype(np.int16),
            "lo_t": lo_t.astype(np.float32),
            "val_t": val_t.astype(np.float32),
            "w1_loc": w1_loc, "w2_loc": w2_loc,
            "th_sq": th_sl.astype(np.float32), "w_sq": w_sq.astype(np.float32),
            "pslot": _wrap16(p_slot.astype(np.int16)),
            "nslot": _wrap16(n_slot.astype(np.int16)),
        }
        for r in range(NUB):
            in_map[f"uidx{r}"] = uidx[r]
            in_map[f"umask{r}"] = umask[r]
        for r in range(NIB):
            in_map[f"pidx{r}"] = pidx[r]
            in_map[f"pmask{r}"] = pmask[r]
            in_map[f"nidx{r}"] = nidx[r]
            in_map[f"nmask{r}"] = nmask[r]
        in_maps.append(in_map)

    return in_maps, S


def _combine(parts_list):
    p = np.sum(np.stack([np.asarray(x, np.float64).reshape(-1) for x in parts_list]),
               axis=0)
    th_sq, w_sq, reg1s, mf, sgp, sgn, gu = p[0], p[1], p[2], p[3], p[4], p[5], p[6]
    UA = -float(np.log(1e-3))
    UB_ = -float(np.log(1.0 - 1e-3))
    mean_mf = mf / (2 * B)
    mean_unknown = (B * (UA + UB_) - (sgp * UA + sgn * UB_)) / (2 * B)
    mean_gu = gu / (2 * B)
    loss = (mean_mf + 0.1 * (mean_unknown - mean_gu)
            + WD * (0.5 * reg1s / B + 0.5 * th_sq / U) + 0.1 * 0.5 * w_sq / I)
    return np.float32(loss)


# ----------------------------------------------------------------------------
# device program
# ----------------------------------------------------------------------------

_BUILD_CACHE = {}


def _build(S):
    if S in _BUILD_CACHE:
        return _BUILD_CACHE[S]
    SEGE = S * 128
    TOTCH = NBLK * NUB * S
    TOTE = TOTCH * 128
    NI = GRP * SEGE

    nc = bacc.Bacc("TRN2", target_bir_lowering=False, num_devices=NCORES)

    theta_d = nc.dram_tensor("theta", [U, C], f32, kind="ExternalInput")
    uemb_d = nc.dram_tensor("uemb", [U, D], f32, kind="ExternalInput")
    iemb_d = nc.dram_tensor("iemb", [I, D], f32, kind="ExternalInput")
    cols16_d = nc.dram_tensor("cols16", [128, TOTE // 16], i16, kind="ExternalInput")
    lo_d = nc.dram_tensor("lo_t", [128, TOTCH], f32, kind="ExternalInput")
    val_d = nc.dram_tensor("val_t", [128, TOTCH], f32, kind="ExternalInput")
    w1l_d = nc.dram_tensor("w1_loc", [128, NBLK], f32, kind="ExternalInput")
    w2l_d = nc.dram_tensor("w2_loc", [128, NBLK], f32, kind="ExternalInput")
    thsq_d = nc.dram_tensor("th_sq", [128, (U // NCORES) * C // 128], f32,
                            kind="ExternalInput")
    wsq_d = nc.dram_tensor("w_sq", [128, 98], f32, kind="ExternalInput")
    uidx_d = [nc.dram_tensor(f"uidx{r}", [128, BPC // 16], i16, kind="ExternalInput")
              for r in range(NUB)]
    umask_d = [nc.dram_tensor(f"umask{r}", [128, NL], f32, kind="ExternalInput")
               for r in range(NUB)]
    pidx_d = [nc.dram_tensor(f"pidx{r}", [128, BPC // 16], i16, kind="ExternalInput")
              for r in range(NIB)]
    pmask_d = [nc.dram_tensor(f"pmask{r}", [128, NL], f32, kind="ExternalInput")
               for r in range(NIB)]
    nidx_d = [nc.dram_tensor(f"nidx{r}", [128, BPC // 16], i16, kind="ExternalInput")
              for r in range(NIB)]
    nmask_d = [nc.dram_tensor(f"nmask{r}", [128, NL], f32, kind="ExternalInput")
               for r in range(NIB)]
    pslot_d = nc.dram_tensor("pslot", [128, BPC // 16], i16, kind="ExternalInput")
    nslot_d = nc.dram_tensor("nslot", [128, BPC // 16], i16, kind="ExternalInput")
    parts_d = nc.dram_tensor("parts", [1, 16], f32, kind="ExternalOutput")

    tpad_d = nc.dram_tensor("tpad", [U, EPAD], f32)
    z1pad_d = nc.dram_tensor("z1pad", [NBLK * 128, EPAD], f32)

    with tile.TileContext(nc) as tc:
        with tc.tile_pool(name="const", bufs=1) as cpool:
            iota_t = cpool.tile([128, 128], f32)
            nc.gpsimd.iota(iota_t[:], pattern=[[1, 128]], base=0,
                           channel_multiplier=0,
                           allow_small_or_imprecise_dtypes=True)
            ones_t = cpool.tile([128, 1], f32)
            nc.vector.memset(ones_t[:], 1.0)
            cols_t = cpool.tile([128, 16], f32)
            nc.vector.memset(cols_t[:], 0.0)

            # ---------------- phase A: softmax -> padded DRAM table ------------
            with tc.tile_pool(name="sm", bufs=3) as sm:
                for t in range(NTT + 1):
                    P = 128 if t < NTT else (U - NTT * 128 * QSM) // QSM
                    rows = slice(t * 128 * QSM, t * 128 * QSM + P * QSM)
                    tin = sm.tile([128, QSM * C], f32, tag="tin")
                    nc.sync.dma_start(
                        out=tin[:P],
                        in_=theta_d[rows, :].rearrange("(p q) c -> p (q c)", q=QSM))
                    te = sm.tile([128, QSM * C], f32, tag="te")
                    nc.scalar.activation(te[:P], tin[:P], AF.Exp)
                    te3 = te[:P].rearrange("p (q c) -> p q c", c=C)
                    ts = sm.tile([128, QSM], f32, tag="ts")
                    nc.vector.reduce_sum(ts[:P], te3, axis=AX.X)
                    tr = sm.tile([128, QSM], f32, tag="tr")
                    nc.vector.reciprocal(tr[:P], ts[:P])
                    tout = sm.tile([128, QSM * EPAD], f32, tag="tout")
                    t3 = tout[:P].rearrange("p (q e) -> p q e", e=EPAD)
                    nc.vector.memset(t3[:, :, C:], 0.0)
                    nc.vector.tensor_tensor(
                        out=t3[:, :, :C], in0=te3,
                        in1=tr[:P, :, None].to_broadcast([P, QSM, C]),
                        op=OP.mult)
                    nc.sync.dma_start(
                        out=tpad_d[rows, :].rearrange("(p q) e -> p (q e)", q=QSM),
                        in_=tout[:P])

                # ---- phase B: reg partials ----
                thsq_t = sm.tile([128, (U // NCORES) * C // 128], f32, tag="thsq")
                nc.sync.dma_start(out=thsq_t[:], in_=thsq_d[:])
                thsq2 = sm.tile([128, (U // NCORES) * C // 128], f32, tag="thsq2")
                nc.vector.tensor_tensor(out=thsq2[:], in0=thsq_t[:], in1=thsq_t[:],
                                        op=OP.mult)
                nc.vector.reduce_sum(cols_t[:, 0:1], thsq2[:], axis=AX.X)
                wsq_t = sm.tile([128, 98], f32, tag="wsq")
                nc.sync.dma_start(out=wsq_t[:], in_=wsq_d[:])
                wsq2 = sm.tile([128, 98], f32, tag="wsq2")
                nc.vector.tensor_tensor(out=wsq2[:], in0=wsq_t[:], in1=wsq_t[:],
                                        op=OP.mult)
                nc.vector.reduce_sum(cols_t[:, 1:2], wsq2[:], axis=AX.X)

            # ---------------- phase C: edges -> z -> z1pad ---------------------
            w1l_t = cpool.tile([128, NBLK], f32)
            w2l_t = cpool.tile([128, NBLK], f32)
            nc.sync.dma_start(out=w1l_t[:], in_=w1l_d[:])
            nc.sync.dma_start(out=w2l_t[:], in_=w2l_d[:])
            lo_full = cpool.tile([128, TOTCH], f32)
            val_full = cpool.tile([128, TOTCH], f32)
            nc.sync.dma_start(out=lo_full[:], in_=lo_d[:])
            nc.sync.dma_start(out=val_full[:], in_=val_d[:])
            c16_full = cpool.tile([128, TOTE // 16], i16)
            nc.sync.dma_start(out=c16_full[:], in_=cols16_d[:])

            with tc.tile_pool(name="main", bufs=1) as mp, \
                 tc.tile_pool(name="mm", bufs=2) as mm, \
                 tc.tile_pool(name="psum", bufs=2, space="PSUM") as pp:
                for g in range(NGRP):
                    gts = []
                    for r in range(NUB):
                        gi = g * NUB + r
                        gt = mp.tile([128, GRP * S * EPAD], f32, tag=f"G{r}")
                        nc.gpsimd.dma_gather(
                            out_ap=gt[:].rearrange("p (s e) -> p s e", e=EPAD),
                            in_ap=tpad_d[r * UBSZ:(r + 1) * UBSZ, :],
                            idxs_ap=c16_full[:, gi * (NI // 16):(gi + 1) * (NI // 16)],
                            num_idxs=NI, num_idxs_reg=NI, elem_size=EPAD,
                            single_packet=False)
                        gt3 = gt[:].rearrange("p (s e) -> p s e", e=EPAD)
                        cbase = gi * GRP * S
                        nc.vector.tensor_tensor(
                            out=gt3[:, :, :C], in0=gt3[:, :, :C],
                            in1=val_full[:, cbase:cbase + GRP * S, None]
                                .to_broadcast([128, GRP * S, C]),
                            op=OP.mult)
                        gts.append(gt)
                    for b_loc in range(GRP):
                        b = g * GRP + b_loc
                        zp = pp.tile([128, C], f32, space="PSUM", tag="zp")
                        for r in range(NUB):
                            cbase = (g * NUB + r) * GRP * S
                            mt = mm.tile([128, S * 128], f32, tag="M")
                            nc.vector.tensor_tensor(
                                out=mt[:].rearrange("p (s q) -> p s q", q=128),
                                in0=lo_full[:, cbase + b_loc * S:
                                            cbase + (b_loc + 1) * S, None]
                                    .to_broadcast([128, S, 128]),
                                in1=iota_t[:, None, :].to_broadcast([128, S, 128]),
                                op=OP.is_equal)
                            g3 = gts[r][:].rearrange("p (s e) -> p s e", e=EPAD)
                            for s in range(S):
                                nc.tensor.matmul(
                                    out=zp[:],
                                    lhsT=mt[:, s * 128:(s + 1) * 128],
                                    rhs=g3[:, b_loc * S + s, :C],
                                    start=(r == 0 and s == 0),
                                    stop=(r == NUB - 1 and s == S - 1))
                        z1s = mm.tile([128, EPAD], f32, tag="z1s")
                        nc.vector.memset(z1s[:, C:], 0.0)
                        nc.scalar.activation(z1s[:, :C], zp[:], AF.Sigmoid,
                                             bias=w2l_t[:, b:b + 1],
                                             scale=w1l_t[:, b:b + 1])
                        nc.sync.dma_start(out=z1pad_d[b * 128:(b + 1) * 128, :],
                                          in_=z1s[:])

            # ---------------- phase D: batch tail ------------------------------
            with tc.tile_pool(name="tail", bufs=2) as tp:
                thu = tp.tile([128, NL * C], f32, tag="thu")
                ue = tp.tile([128, NL * D], f32, tag="ue")
                pe = tp.tile([128, NL * D], f32, tag="pe")
                ne = tp.tile([128, NL * D], f32, tag="ne")
                for t_ in (thu, ue, pe, ne):
                    nc.vector.memset(t_[:], 0.0)

                def masked_gather(acc3, table_view, idx_d_, mask_d_, width):
                    gtile = tp.tile([128, NL * EPAD], f32, tag="bg")
                    itile = tp.tile([128, BPC // 16], i16, tag="bidx")
                    mtile = tp.tile([128, NL], f32, tag="bmask")
                    nc.sync.dma_start(out=itile[:], in_=idx_d_[:])
                    nc.sync.dma_start(out=mtile[:], in_=mask_d_[:])
                    nc.gpsimd.dma_gather(
                        out_ap=gtile[:].rearrange("p (s e) -> p s e", e=EPAD),
                        in_ap=table_view,
                        idxs_ap=itile[:], num_idxs=BPC, num_idxs_reg=BPC,
                        elem_size=EPAD, single_packet=False)
                    g3 = gtile[:].rearrange("p (s e) -> p s e", e=EPAD)
                    tmp = tp.tile([128, NL * width], f32, tag="btmp")
                    tmp3 = tmp[:].rearrange("p (s e) -> p s e", e=width)
                    nc.vector.tensor_tensor(
                        out=tmp3, in0=g3[:, :, :width],
                        in1=mtile[:, :, None].to_broadcast([128, NL, width]),
                        op=OP.mult)
                    nc.vector.tensor_tensor(out=acc3, in0=acc3, in1=tmp3, op=OP.add)

                thu3 = thu[:].rearrange("p (s c) -> p s c", c=C)
                ue3 = ue[:].rearrange("p (s d) -> p s d", d=D)
                pe3 = pe[:].rearrange("p (s d) -> p s d", d=D)
                ne3 = ne[:].rearrange("p (s d) -> p s d", d=D)
                for r in range(NUB):
                    masked_gather(thu3, tpad_d[r * UBSZ:(r + 1) * UBSZ, :],
                                  uidx_d[r], umask_d[r], C)
                    masked_gather(ue3, uemb_d[r * UBSZ:(r + 1) * UBSZ, :],
                                  uidx_d[r], umask_d[r], D)
                for r in range(NIB):
                    sz = min(IBSZ, I - r * IBSZ)
                    masked_gather(pe3, iemb_d[r * IBSZ:r * IBSZ + sz, :],
                                  pidx_d[r], pmask_d[r], D)
                    masked_gather(ne3, iemb_d[r * IBSZ:r * IBSZ + sz, :],
                                  nidx_d[r], nmask_d[r], D)

                def plain_gather(slot_d_):
                    gtile = tp.tile([128, NL * EPAD], f32, tag="zg")
                    itile = tp.tile([128, BPC // 16], i16, tag="zidx")
                    nc.sync.dma_start(out=itile[:], in_=slot_d_[:])
                    nc.gpsimd.dma_gather(
                        out_ap=gtile[:].rearrange("p (s e) -> p s e", e=EPAD),
                        in_ap=z1pad_d[:],
                        idxs_ap=itile[:], num_idxs=BPC, num_idxs_reg=BPC,
                        elem_size=EPAD, single_packet=False)
                    return gtile[:].rearrange("p (s e) -> p s e", e=EPAD)

                z1p3 = plain_gather(pslot_d)
                z1n3 = plain_gather(nslot_d)

                def rowdot(in0_3, in1_3, width, tag):
                    prod = tp.tile([128, NL * width], f32, tag=f"{tag}_p")
                    p3 = prod[:].rearrange("p (s e) -> p s e", e=width)
                    nc.vector.tensor_tensor(out=p3, in0=in0_3, in1=in1_3, op=OP.mult)
                    out = tp.tile([128, NL], f32, tag=f"{tag}_r")
                    nc.vector.reduce_sum(out[:], p3, axis=AX.X)
                    return out

                s_pos = rowdot(ue3, pe3, D, "sp")
                s_neg = rowdot(ue3, ne3, D, "sn")
                g_pos = rowdot(thu3, z1p3[:, :, :C], C, "gp")
                g_neg = rowdot(thu3, z1n3[:, :, :C], C, "gn")

                # bce_pos = -clip(ln sigmoid(s_pos)); bce_neg = -clip(ln sigmoid(-s_neg))
                sigp = tp.tile([128, NL], f32, tag="sigp")
                nc.scalar.activation(sigp[:], s_pos[:], AF.Sigmoid)
                lsp = tp.tile([128, NL], f32, tag="lsp")
                nc.scalar.activation(lsp[:], sigp[:], AF.Ln)
                nc.vector.tensor_scalar(out=lsp[:], in0=lsp[:], scalar1=-100.0,
                                        scalar2=None, op0=OP.max)
                sign_ = tp.tile([128, NL], f32, tag="sign")
                nc.scalar.activation(sign_[:], s_neg[:], AF.Sigmoid, scale=-1.0)
                lsn = tp.tile([128, NL], f32, tag="lsn")
                nc.scalar.activation(lsn[:], sign_[:], AF.Ln)
                nc.vector.tensor_scalar(out=lsn[:], in0=lsn[:], scalar1=-100.0,
                                        scalar2=None, op0=OP.max)

                mf1 = tp.tile([128, NL], f32, tag="mf1")
                nc.vector.tensor_tensor(out=mf1[:], in0=g_pos[:], in1=lsp[:],
                                        op=OP.mult)
                mf2 = tp.tile([128, NL], f32, tag="mf2")
                nc.vector.tensor_tensor(out=mf2[:], in0=g_neg[:], in1=lsn[:],
                                        op=OP.mult)
                nc.vector.tensor_tensor(out=mf1[:], in0=mf1[:], in1=mf2[:], op=OP.add)
                nc.vector.tensor_scalar(out=mf1[:], in0=mf1[:], scalar1=-1.0,
                                        scalar2=None, op0=OP.mult)
                nc.vector.reduce_sum(cols_t[:, 3:4], mf1[:], axis=AX.X)

                nc.vector.reduce_sum(cols_t[:, 4:5], g_pos[:], axis=AX.X)
                nc.vector.reduce_sum(cols_t[:, 5:6], g_neg[:], axis=AX.X)

                # gamma uncertainty: -(g ln g + (1-g) ln(1-g)), logs clamped at -100
                gu_acc = tp.tile([128, NL], f32, tag="gu")
                nc.vector.memset(gu_acc[:], 0.0)
                for gg in (g_pos, g_neg):
                    lg = tp.tile([128, NL], f32, tag="lg")
                    nc.scalar.activation(lg[:], gg[:], AF.Ln)
                    nc.vector.tensor_scalar(out=lg[:], in0=lg[:], scalar1=-100.0,
                                            scalar2=None, op0=OP.max)
                    omg = tp.tile([128, NL], f32, tag="omg")
                    nc.vector.tensor_scalar(out=omg[:], in0=gg[:], scalar1=-1.0,
                                            scalar2=1.0, op0=OP.mult, op1=OP.add)
                    l1g = tp.tile([128, NL], f32, tag="l1g")
                    nc.scalar.activation(l1g[:], omg[:], AF.Ln)
                    nc.vector.tensor_scalar(out=l1g[:], in0=l1g[:], scalar1=-100.0,
                                            scalar2=None, op0=OP.max)
                    t1 = tp.tile([128, NL], f32, tag="gu_t1")
                    nc.vector.tensor_tensor(out=t1[:], in0=gg[:], in1=lg[:], op=OP.mult)
                    t2 = tp.tile([128, NL], f32, tag="gu_t2")
                    nc.vector.tensor_tensor(out=t2[:], in0=omg[:], in1=l1g[:],
                                            op=OP.mult)
                    nc.vector.tensor_tensor(out=t1[:], in0=t1[:], in1=t2[:], op=OP.add)
                    nc.vector.tensor_tensor(out=gu_acc[:], in0=gu_acc[:], in1=t1[:],
                                            op=OP.add)
                nc.vector.tensor_scalar(out=gu_acc[:], in0=gu_acc[:], scalar1=-1.0,
                                        scalar2=None, op0=OP.mult)
                nc.vector.reduce_sum(cols_t[:, 6:7], gu_acc[:], axis=AX.X)

                # reg1: sum of squares of gathered embeddings
                r1 = tp.tile([128, 1], f32, tag="r1")
                nc.vector.memset(r1[:], 0.0)
                for emb in (ue, pe, ne):
                    sq = tp.tile([128, NL * D], f32, tag="r1sq")
                    nc.vector.tensor_tensor(out=sq[:], in0=emb[:], in1=emb[:],
                                            op=OP.mult)
                    rs = tp.tile([128, 1], f32, tag="r1rs")
                    nc.vector.reduce_sum(rs[:], sq[:], axis=AX.X)
                    nc.vector.tensor_tensor(out=r1[:], in0=r1[:], in1=rs[:], op=OP.add)
                nc.vector.tensor_copy(out=cols_t[:, 2:3], in_=r1[:])

                # final partition reduction via ones-matmul
                with tc.tile_pool(name="fps", bufs=1, space="PSUM") as fp:
                    pout = fp.tile([1, 16], f32, space="PSUM")
                    nc.tensor.matmul(out=pout[:], lhsT=ones_t[:], rhs=cols_t[:],
                                     start=True, stop=True)
                    pres = tp.tile([1, 16], f32, tag="pres")
                    nc.vector.tensor_copy(out=pres[:], in_=pout[:])
                    nc.sync.dma_start(out=parts_d[:], in_=pres[:])

    nc.compile()
    _BUILD_CACHE[S] = nc
    return nc


def run(inputs, trace=False, trace_kwargs=None):
    from concourse.bass_utils import run_bass_kernel_spmd
    in_maps, S = _prep(inputs)
    nc = _build(S)
    kw = {}
    if trace:
        kw["trace"] = True
        if trace_kwargs:
            kw.update(trace_kwargs)
    res = run_bass_kernel_spmd(nc, in_maps, core_ids=list(range(NCORES)), **kw)
    parts = [res.results[k]["parts"] for k in range(NCORES)]
    loss = _combine(parts)
    return loss, res


def kernel(**inputs) -> np.ndarray:
    loss, _ = run(inputs)
    return np.asarray(loss, dtype=np.float32)


# revision 5
# speedup vs baseline: 1.6903x; 1.6903x over previous
"""Trainium2 Bass kernel for the FAWMF loss (gnn_message_passing).

Strategy (8 NeuronCores, SPMD, no collectives):
  - Only item-rows of z matter, and only for items referenced by the batch
    (all_theta[col]=0 for col>=U, z[:U] is discarded by the reference).
  - Each core takes 1/8 of the (users, pos, neg) batch plus exactly the edges
    feeding its batch's items (need-based edge sharding). Items are remapped to
    <=4096 local slots (32 blocks of 128), balanced by degree.
  - Device: softmax(theta_user) is computed on-chip and written to a padded
    bf16 [U, 128] DRAM table (256B rows). Edge cols are gathered from it with
    GPSIMD dma_gather (int16 indices, 4 range-buckets, 4 SWDGE queues), scaled
    by edge vals, and segment-summed into z blocks via one-hot bf16 matmuls
    accumulating in fp32 PSUM. z1 = sigmoid(w1*z + w2) via ACT with
    per-partition scale/bias, written to a bf16 DRAM table; batch rows of
    z1/theta/embeddings are gathered back and reduced to 7 partial sums.
  - Host combines the 8 partial-sum vectors into the scalar loss.
"""
import os
import sys
import numpy as np

for _p in ("/opt/trn_rl_repo", "/root/.axon_site", "/root/.axon_site/_ro/trn_rl_repo",
           "/root/.axon_site/_ro/pypackages"):
    if os.path.isdir(_p) and _p not in sys.path:
        sys.path.append(_p)

import ml_dtypes
import concourse.bacc as bacc
import concourse.bass as bass
import concourse.mybir as mybir
import concourse.tile as tile

f32 = mybir.dt.float32
bf16 = mybir.dt.bfloat16
i16 = mybir.dt.int16
AF = mybir.ActivationFunctionType
OP = mybir.AluOpType
AX = mybir.AxisListType
np_bf16 = ml_dtypes.bfloat16

U, I, C, D, B = 100000, 50000, 32, 64, 16384
NCORES = 8
BPC = B // NCORES
NL = BPC // 128             # batch lanes per partition (16)
NBLK = 32                   # 128-row z blocks per core
NUB = 4                     # user-id buckets (int16 gather)
UBSZ = 25000
NIB = 2                     # item-id buckets
IBSZ = 32768
EPAD = 128                  # padded bf16 table row width (256B)
GRP = 4                     # blocks per gather group
NGRP = NBLK // GRP
NTT = 15                    # full softmax tiles (128 part x 50 rows)
QSM = 50                    # user rows per partition per softmax tile
WD = 1e-4
NQ = 4                      # SWDGE queues

# ----------------------------------------------------------------------------
# host-side sharding helpers
# ----------------------------------------------------------------------------

def _wrap16(idx):
    n = idx.shape[0]
    a = idx.reshape(n // 16, 16).T.astype(np.int16)
    return np.ascontiguousarray(np.tile(a, (8, 1)))


def _lanes(a, ncol):
    return np.ascontiguousarray(a.reshape(ncol, 128).T)


def _prep(inputs):
    users = np.asarray(inputs["users"]).astype(np.int64)
    pos = np.asarray(inputs["positive_items"]).astype(np.int64)
    neg = np.asarray(inputs["negative_items"]).astype(np.int64)
    er_full = np.asarray(inputs["edge_rows"])
    ec_full = np.asarray(inputs["edge_cols"])
    ev_full = np.asarray(inputs["edge_vals"])
    w1 = np.asarray(inputs["w1"], np.float32).reshape(-1)
    w2 = np.asarray(inputs["w2"], np.float32).reshape(-1)
    theta = np.asarray(inputs["theta_user"], np.float32)
    uemb = np.asarray(inputs["user_embedding"], np.float32)
    iemb = np.asarray(inputs["item_embedding"], np.float32)

    m = (er_full >= U) & (ec_full < U)
    er = (er_full[m] - U).astype(np.int64)
    ec = ec_full[m].astype(np.int64)
    ev = np.asarray(ev_full[m], np.float32)
    order = np.argsort(er, kind="stable")
    er, ec, ev = er[order], ec[order], ev[order]
    counts = np.bincount(er, minlength=I)
    ptr = np.zeros(I + 1, np.int64)
    np.cumsum(counts, out=ptr[1:])

    import heapq
    cores = []
    S_needed = 1
    for k in range(NCORES):
        u_k = users[k * BPC:(k + 1) * BPC]
        p_k = pos[k * BPC:(k + 1) * BPC]
        n_k = neg[k * BPC:(k + 1) * BPC]
        ids = np.unique(np.concatenate([p_k, n_k]))
        deg = counts[ids]
        o = np.argsort(-deg, kind="stable")
        blk_of = np.empty(len(ids), np.int64)
        slot_in_blk = np.empty(len(ids), np.int64)
        heap = [(0, 0, b) for b in range(NBLK)]
        heapq.heapify(heap)
        for i_ in o:
            while True:
                load, cnt, b = heapq.heappop(heap)
                if cnt < 128:
                    break
            blk_of[i_] = b
            slot_in_blk[i_] = cnt
            heapq.heappush(heap, (load + int(deg[i_]), cnt + 1, b))
        slot = blk_of * 128 + slot_in_blk
        slot_of_item = np.full(I, -1, np.int64)
        slot_of_item[ids] = slot

        lens = counts[ids]
        tot = int(lens.sum())
        startp = ptr[ids]
        idx = np.repeat(startp + lens - np.cumsum(lens), lens) + np.arange(tot)
        e_col = ec[idx]
        e_val = ev[idx]
        e_slot = np.repeat(slot, lens)
        e_lo = (e_slot & 127).astype(np.float32)
        e_bkt = e_col // UBSZ
        seg_key = (e_slot >> 7) * NUB + e_bkt
        seg_cnt = np.bincount(seg_key, minlength=NBLK * NUB)
        S_needed = max(S_needed, int(np.ceil(seg_cnt.max() / 128)))
        cores.append(dict(u_k=u_k, p_k=p_k, n_k=n_k, ids=ids, slot=slot,
                          slot_of_item=slot_of_item, e_col=e_col, e_val=e_val,
                          e_lo=e_lo, seg_key=seg_key))

    S = S_needed
    SEGE = S * 128
    TOTCH = NBLK * NUB * S
    TOTE = TOTCH * 128
    NI = GRP * SEGE

    seg_order = [(g * GRP + b_loc, r)
                 for g in range(NGRP) for r in range(NUB) for b_loc in range(GRP)]

    in_maps = []
    for k in range(NCORES):
        c = cores[k]
        segs_cols = np.zeros(TOTE, np.int64)
        segs_vals = np.zeros(TOTE, np.float32)
        segs_lo = np.zeros(TOTE, np.float32)
        sort_by_seg = np.argsort(c["seg_key"], kind="stable")
        seg_sorted_cols = c["e_col"][sort_by_seg]
        seg_sorted_vals = c["e_val"][sort_by_seg]
        seg_sorted_lo = c["e_lo"][sort_by_seg]
        seg_cnt = np.bincount(c["seg_key"], minlength=NBLK * NUB)
        seg_ptr = np.zeros(NBLK * NUB + 1, np.int64)
        np.cumsum(seg_cnt, out=seg_ptr[1:])
        off = 0
        for (b, r) in seg_order:
            sk = b * NUB + r
            ns = int(seg_cnt[sk])
            sl = slice(seg_ptr[sk], seg_ptr[sk] + ns)
            segs_cols[off:off + ns] = seg_sorted_cols[sl]
            segs_vals[off:off + ns] = seg_sorted_vals[sl]
            segs_lo[off:off + ns] = seg_sorted_lo[sl]
            segs_cols[off + ns:off + SEGE] = r * UBSZ
            off += SEGE

        cols16_parts = []
        for gi in range(NGRP * NUB):
            r = gi % NUB
            rel = segs_cols[gi * NI:(gi + 1) * NI] - r * UBSZ
            cols16_parts.append(_wrap16(rel.astype(np.int16)))
        cols16 = np.concatenate(cols16_parts, axis=1)

        lo_t = _lanes(segs_lo, TOTCH).astype(np_bf16)
        val_t = _lanes(segs_vals, TOTCH).astype(np_bf16)

        som = c["slot_of_item"]
        p_slot = som[c["p_k"]]
        n_slot = som[c["n_k"]]

        def bucketize(idx_global, nb, bsz):
            arrs, masks = [], []
            for r in range(nb):
                inb = (idx_global >= r * bsz) & (idx_global < (r + 1) * bsz)
                rel = np.where(inb, idx_global - r * bsz, 0)
                arrs.append(_wrap16(rel.astype(np.int16)))
                masks.append(_lanes(inb.astype(np.float32), NL))
            return arrs, masks

        uidx, umask = bucketize(c["u_k"], NUB, UBSZ)
        pidx, pmask = bucketize(c["p_k"], NIB, IBSZ)
        nidx, nmask = bucketize(c["n_k"], NIB, IBSZ)

        w1_loc = np.zeros((128, NBLK), np.float32)
        w2_loc = np.zeros((128, NBLK), np.float32)
        slot = c["slot"]
        w1_loc[slot & 127, slot >> 7] = w1[c["ids"]]
        w2_loc[slot & 127, slot >> 7] = w2[c["ids"]]

        th_sl = np.ascontiguousarray(
            theta[k * (U // NCORES):(k + 1) * (U // NCORES)].reshape(128, -1))
        wcat = np.concatenate([w1[k * (I // NCORES):(k + 1) * (I // NCORES)],
                               w2[k * (I // NCORES):(k + 1) * (I // NCORES)]])
        wpad = np.zeros(128 * 98, np.float32)
        wpad[:len(wcat)] = wcat
        w_sq = np.ascontiguousarray(wpad.reshape(98, 128).T)

        in_map = {
            "theta": theta, "uemb": uemb, "iemb": iemb,
            "cols16": cols16.astype(np.int16),
            "lo_t": lo_t, "val_t": val_t,
            "w1_loc": w1_loc, "w2_loc": w2_loc,
            "th_sq": th_sl.astype(np.float32), "w_sq": w_sq.astype(np.float32),
            "pslot": _wrap16(p_slot.astype(np.int16)),
            "nslot": _wrap16(n_slot.astype(np.int16)),
        }
        for r in range(NUB):
            in_map[f"uidx{r}"] = uidx[r]
            in_map[f"umask{r}"] = umask[r]
        for r in range(NIB):
            in_map[f"pidx{r}"] = pidx[r]
            in_map[f"pmask{r}"] = pmask[r]
            in_map[f"nidx{r}"] = nidx[r]
            in_map[f"nmask{r}"] = nmask[r]
        in_maps.append(in_map)

    return in_maps, S


def _combine(parts_list):
    p = np.sum(np.stack([np.asarray(x, np.float64).reshape(-1) for x in parts_list]),
               axis=0)
    th_sq, w_sq, reg1s, mf, sgp, sgn, gu = p[0], p[1], p[2], p[3], p[4], p[5], p[6]
    UA = -float(np.log(1e-3))
    UB_ = -float(np.log(1.0 - 1e-3))
    mean_mf = mf / (2 * B)
    mean_unknown = (B * (UA + UB_) - (sgp * UA + sgn * UB_)) / (2 * B)
    mean_gu = gu / (2 * B)
    loss = (mean_mf + 0.1 * (mean_unknown - mean_gu)
            + WD * (0.5 * reg1s / B + 0.5 * th_sq / U) + 0.1 * 0.5 * w_sq / I)
    return np.float32(loss)


# ----------------------------------------------------------------------------
# device program
# ----------------------------------------------------------------------------

_BUILD_CACHE = {}


def _build(S):
    if S in _BUILD_CACHE:
        return _BUILD_CACHE[S]
    SEGE = S * 128
    TOTCH = NBLK * NUB * S
    TOTE = TOTCH * 128
    NI = GRP * SEGE

    nc = bacc.Bacc("TRN2", target_bir_lowering=False, num_devices=NCORES,
                   num_swdge_queues=NQ)

    theta_d = nc.dram_tensor("theta", [U, C], f32, kind="ExternalInput")
    uemb_d = nc.dram_tensor("uemb", [U, D], f32, kind="ExternalInput")
    iemb_d = nc.dram_tensor("iemb", [I, D], f32, kind="ExternalInput")
    cols16_d = nc.dram_tensor("cols16", [128, TOTE // 16], i16, kind="ExternalInput")
    lo_d = nc.dram_tensor("lo_t", [128, TOTCH], bf16, kind="ExternalInput")
    val_d = nc.dram_tensor("val_t", [128, TOTCH], bf16, kind="ExternalInput")
    w1l_d = nc.dram_tensor("w1_loc", [128, NBLK], f32, kind="ExternalInput")
    w2l_d = nc.dram_tensor("w2_loc", [128, NBLK], f32, kind="ExternalInput")
    thsq_d = nc.dram_tensor("th_sq", [128, (U // NCORES) * C // 128], f32,
                            kind="ExternalInput")
    wsq_d = nc.dram_tensor("w_sq", [128, 98], f32, kind="ExternalInput")
    uidx_d = [nc.dram_tensor(f"uidx{r}", [128, BPC // 16], i16, kind="ExternalInput")
              for r in range(NUB)]
    umask_d = [nc.dram_tensor(f"umask{r}", [128, NL], f32, kind="ExternalInput")
               for r in range(NUB)]
    pidx_d = [nc.dram_tensor(f"pidx{r}", [128, BPC // 16], i16, kind="ExternalInput")
              for r in range(NIB)]
    pmask_d = [nc.dram_tensor(f"pmask{r}", [128, NL], f32, kind="ExternalInput")
               for r in range(NIB)]
    nidx_d = [nc.dram_tensor(f"nidx{r}", [128, BPC // 16], i16, kind="ExternalInput")
              for r in range(NIB)]
    nmask_d = [nc.dram_tensor(f"nmask{r}", [128, NL], f32, kind="ExternalInput")
               for r in range(NIB)]
    pslot_d = nc.dram_tensor("pslot", [128, BPC // 16], i16, kind="ExternalInput")
    nslot_d = nc.dram_tensor("nslot", [128, BPC // 16], i16, kind="ExternalInput")
    parts_d = nc.dram_tensor("parts", [1, 16], f32, kind="ExternalOutput")

    tpad_d = nc.dram_tensor("tpad", [U, EPAD], bf16)
    z1pad_d = nc.dram_tensor("z1pad", [NBLK * 128, EPAD], bf16)

    qrr = [0]

    def next_q():
        q = qrr[0]
        qrr[0] = (q + 1) % NQ
        return q

    with tile.TileContext(nc) as tc:
        with tc.tile_pool(name="const", bufs=1) as cpool, \
             tc.tile_pool(name="tail", bufs=1) as tp:
            iota_t = cpool.tile([128, 128], bf16)
            nc.gpsimd.iota(iota_t[:], pattern=[[1, 128]], base=0,
                           channel_multiplier=0,
                           allow_small_or_imprecise_dtypes=True)
            ones_t = cpool.tile([128, 1], f32)
            nc.vector.memset(ones_t[:], 1.0)
            cols_t = cpool.tile([128, 16], f32)
            nc.vector.memset(cols_t[:], 0.0)

            # ---- batch embedding gathers (independent of softmax) ----
            ue = tp.tile([128, NL * D], f32, tag="ue")
            pe = tp.tile([128, NL * D], f32, tag="pe")
            ne = tp.tile([128, NL * D], f32, tag="ne")
            for t_ in (ue, pe, ne):
                nc.vector.memset(t_[:], 0.0)
            ue3 = ue[:].rearrange("p (s d) -> p s d", d=D)
            pe3 = pe[:].rearrange("p (s d) -> p s d", d=D)
            ne3 = ne[:].rearrange("p (s d) -> p s d", d=D)

            def masked_gather_f32(acc3, table_view, idx_d_, mask_d_):
                gtile = tp.tile([128, NL * D], f32, tag="bg")
                itile = tp.tile([128, BPC // 16], i16, tag="bidx")
                mtile = tp.tile([128, NL], f32, tag="bmask")
                nc.sync.dma_start(out=itile[:], in_=idx_d_[:])
                nc.sync.dma_start(out=mtile[:], in_=mask_d_[:])
                nc.gpsimd.dma_gather(
                    out_ap=gtile[:].rearrange("p (s e) -> p s e", e=D),
                    in_ap=table_view,
                    idxs_ap=itile[:], num_idxs=BPC, num_idxs_reg=BPC,
                    elem_size=D, single_packet=False, queue_num=next_q())
                g3 = gtile[:].rearrange("p (s e) -> p s e", e=D)
                tmp = tp.tile([128, NL * D], f32, tag="btmp")
                tmp3 = tmp[:].rearrange("p (s e) -> p s e", e=D)
                nc.vector.tensor_tensor(
                    out=tmp3, in0=g3,
                    in1=mtile[:, :, None].to_broadcast([128, NL, D]),
                    op=OP.mult)
                nc.vector.tensor_tensor(out=acc3, in0=acc3, in1=tmp3, op=OP.add)

            for r in range(NUB):
                masked_gather_f32(ue3, uemb_d[r * UBSZ:(r + 1) * UBSZ, :],
                                  uidx_d[r], umask_d[r])
            for r in range(NIB):
                sz = min(IBSZ, I - r * IBSZ)
                masked_gather_f32(pe3, iemb_d[r * IBSZ:r * IBSZ + sz, :],
                                  pidx_d[r], pmask_d[r])
                masked_gather_f32(ne3, iemb_d[r * IBSZ:r * IBSZ + sz, :],
                                  nidx_d[r], nmask_d[r])

            # scores + bce + reg1 (independent of softmax table)
            def rowdot(in0_3, in1_3, width, tag, odt=f32):
                prod = tp.tile([128, NL * width], odt, tag=f"{tag}_p")
                p3 = prod[:].rearrange("p (s e) -> p s e", e=width)
                nc.vector.tensor_tensor(out=p3, in0=in0_3, in1=in1_3, op=OP.mult)
                out = tp.tile([128, NL], f32, tag=f"{tag}_r")
                nc.vector.reduce_sum(out[:], p3, axis=AX.X)
                return out

            s_pos = rowdot(ue3, pe3, D, "sp")
            s_neg = rowdot(ue3, ne3, D, "sn")

            sigp = tp.tile([128, NL], f32, tag="sigp")
            nc.scalar.activation(sigp[:], s_pos[:], AF.Sigmoid)
            lsp = tp.tile([128, NL], f32, tag="lsp")
            nc.scalar.activation(lsp[:], sigp[:], AF.Ln)
            nc.vector.tensor_scalar(out=lsp[:], in0=lsp[:], scalar1=-100.0,
                                    scalar2=None, op0=OP.max)
            sign_ = tp.tile([128, NL], f32, tag="sign")
            nc.scalar.activation(sign_[:], s_neg[:], AF.Sigmoid, scale=-1.0)
            lsn = tp.tile([128, NL], f32, tag="lsn")
            nc.scalar.activation(lsn[:], sign_[:], AF.Ln)
            nc.vector.tensor_scalar(out=lsn[:], in0=lsn[:], scalar1=-100.0,
                                    scalar2=None, op0=OP.max)

            r1 = tp.tile([128, 1], f32, tag="r1")
            nc.vector.memset(r1[:], 0.0)
            for emb in (ue, pe, ne):
                sq = tp.tile([128, NL * D], f32, tag="r1sq")
                nc.vector.tensor_tensor(out=sq[:], in0=emb[:], in1=emb[:],
                                        op=OP.mult)
                rs = tp.tile([128, 1], f32, tag="r1rs")
                nc.vector.reduce_sum(rs[:], sq[:], axis=AX.X)
                nc.vector.tensor_tensor(out=r1[:], in0=r1[:], in1=rs[:], op=OP.add)
            nc.vector.tensor_copy(out=cols_t[:, 2:3], in_=r1[:])

            # ---------------- phase A: softmax -> padded bf16 DRAM table -------
            with tc.tile_pool(name="sm", bufs=2) as sm:
                for t in range(NTT + 1):
                    P = 128 if t < NTT else (U - NTT * 128 * QSM) // QSM
                    rows = slice(t * 128 * QSM, t * 128 * QSM + P * QSM)
                    tin = sm.tile([128, QSM * C], f32, tag="tin")
                    nc.sync.dma_start(
                        out=tin[:P],
                        in_=theta_d[rows, :].rearrange("(p q) c -> p (q c)", q=QSM))
                    te = sm.tile([128, QSM * C], f32, tag="te")
                    nc.scalar.activation(te[:P], tin[:P], AF.Exp)
                    te3 = te[:P].rearrange("p (q c) -> p q c", c=C)
                    ts = sm.tile([128, QSM], f32, tag="ts")
                    nc.vector.reduce_sum(ts[:P], te3, axis=AX.X)
                    tr = sm.tile([128, QSM], f32, tag="tr")
                    nc.vector.reciprocal(tr[:P], ts[:P])
                    tout = sm.tile([128, QSM * EPAD], bf16, tag="tout")
                    t3 = tout[:P].rearrange("p (q e) -> p q e", e=EPAD)
                    nc.vector.memset(t3[:, :, C:], 0.0)
                    nc.vector.tensor_tensor(
                        out=t3[:, :, :C], in0=te3,
                        in1=tr[:P, :, None].to_broadcast([P, QSM, C]),
                        op=OP.mult)
                    nc.sync.dma_start(
                        out=tpad_d[rows, :].rearrange("(p q) e -> p (q e)", q=QSM),
                        in_=tout[:P])

            # ---- reg partials ----
            with tc.tile_pool(name="regs", bufs=1) as rp:
                thsq_t = rp.tile([128, (U // NCORES) * C // 128], f32, tag="thsq")
                nc.sync.dma_start(out=thsq_t[:], in_=thsq_d[:])
                thsq2 = rp.tile([128, (U // NCORES) * C // 128], f32, tag="thsq2")
                nc.vector.tensor_tensor(out=thsq2[:], in0=thsq_t[:], in1=thsq_t[:],
                                        op=OP.mult)
                nc.vector.reduce_sum(cols_t[:, 0:1], thsq2[:], axis=AX.X)
                wsq_t = rp.tile([128, 98], f32, tag="wsq")
                nc.sync.dma_start(out=wsq_t[:], in_=wsq_d[:])
                wsq2 = rp.tile([128, 98], f32, tag="wsq2")
                nc.vector.tensor_tensor(out=wsq2[:], in0=wsq_t[:], in1=wsq_t[:],
                                        op=OP.mult)
                nc.vector.reduce_sum(cols_t[:, 1:2], wsq2[:], axis=AX.X)

            # ---------------- phase C: edges -> z -> z1pad ---------------------
            w1l_t = cpool.tile([128, NBLK], f32)
            w2l_t = cpool.tile([128, NBLK], f32)
            nc.sync.dma_start(out=w1l_t[:], in_=w1l_d[:])
            nc.sync.dma_start(out=w2l_t[:], in_=w2l_d[:])
            lo_full = cpool.tile([128, TOTCH], bf16)
            val_full = cpool.tile([128, TOTCH], bf16)
            nc.sync.dma_start(out=lo_full[:], in_=lo_d[:])
            nc.sync.dma_start(out=val_full[:], in_=val_d[:])
            c16_full = cpool.tile([128, TOTE // 16], i16)
            nc.sync.dma_start(out=c16_full[:], in_=cols16_d[:])

            with tc.tile_pool(name="main", bufs=1) as mp, \
                 tc.tile_pool(name="mm", bufs=2) as mm, \
                 tc.tile_pool(name="psum", bufs=2, space="PSUM") as pp:
                for g in range(NGRP):
                    gts = []
                    for r in range(NUB):
                        gi = g * NUB + r
                        gt = mp.tile([128, GRP * S * EPAD], bf16, tag=f"G{r}")
                        nc.gpsimd.dma_gather(
                            out_ap=gt[:].rearrange("p (s e) -> p s e", e=EPAD),
                            in_ap=tpad_d[r * UBSZ:(r + 1) * UBSZ, :],
                            idxs_ap=c16_full[:, gi * (NI // 16):(gi + 1) * (NI // 16)],
                            num_idxs=NI, num_idxs_reg=NI, elem_size=EPAD,
                            single_packet=False, queue_num=r)
                        gt3 = gt[:].rearrange("p (s e) -> p s e", e=EPAD)
                        cbase = gi * GRP * S
                        nc.vector.tensor_tensor(
                            out=gt3[:, :, :C], in0=gt3[:, :, :C],
                            in1=val_full[:, cbase:cbase + GRP * S, None]
                                .to_broadcast([128, GRP * S, C]),
                            op=OP.mult)
                        gts.append(gt)
                    for b_loc in range(GRP):
                        b = g * GRP + b_loc
                        zp = pp.tile([128, C], f32, space="PSUM", tag="zp")
                        for r in range(NUB):
                            cbase = (g * NUB + r) * GRP * S
                            mt = mm.tile([128, S * 128], bf16, tag="M")
                            nc.vector.tensor_tensor(
                                out=mt[:].rearrange("p (s q) -> p s q", q=128),
                                in0=lo_full[:, cbase + b_loc * S:
                                            cbase + (b_loc + 1) * S, None]
                                    .to_broadcast([128, S, 128]),
                                in1=iota_t[:, None, :].to_broadcast([128, S, 128]),
                                op=OP.is_equal)
                            g3 = gts[r][:].rearrange("p (s e) -> p s e", e=EPAD)
                            for s in range(S):
                                nc.tensor.matmul(
                                    out=zp[:],
                                    lhsT=mt[:, s * 128:(s + 1) * 128],
                                    rhs=g3[:, b_loc * S + s, :C],
                                    start=(r == 0 and s == 0),
                                    stop=(r == NUB - 1 and s == S - 1))
                        z1s = mm.tile([128, EPAD], bf16, tag="z1s")
                        nc.vector.memset(z1s[:, C:], 0.0)
                        nc.scalar.activation(z1s[:, :C], zp[:], AF.Sigmoid,
                                             bias=w2l_t[:, b:b + 1],
                                             scale=w1l_t[:, b:b + 1])
                        nc.sync.dma_start(out=z1pad_d[b * 128:(b + 1) * 128, :],
                                          in_=z1s[:])

            # ---------------- phase D: softmax-dependent batch tail ------------
            thu = tp.tile([128, NL * C], bf16, tag="thu")
            nc.vector.memset(thu[:], 0.0)
            thu3 = thu[:].rearrange("p (s c) -> p s c", c=C)
            for r in range(NUB):
                gtile = tp.tile([128, NL * EPAD], bf16, tag="tg")
                itile = tp.tile([128, BPC // 16], i16, tag="tgidx")
                mtile = tp.tile([128, NL], bf16, tag="tgmask")
                mtile_f = tp.tile([128, NL], f32, tag="tgmaskf")
                nc.sync.dma_start(out=itile[:], in_=uidx_d[r][:])
                nc.sync.dma_start(out=mtile_f[:], in_=umask_d[r][:])
                nc.vector.tensor_copy(out=mtile[:], in_=mtile_f[:])
                nc.gpsimd.dma_gather(
                    out_ap=gtile[:].rearrange("p (s e) -> p s e", e=EPAD),
                    in_ap=tpad_d[r * UBSZ:(r + 1) * UBSZ, :],
                    idxs_ap=itile[:], num_idxs=BPC, num_idxs_reg=BPC,
                    elem_size=EPAD, single_packet=False, queue_num=next_q())
                g3 = gtile[:].rearrange("p (s e) -> p s e", e=EPAD)
                tmp = tp.tile([128, NL * C], bf16, tag="ttmp")
                tmp3 = tmp[:].rearrange("p (s e) -> p s e", e=C)
                nc.vector.tensor_tensor(
                    out=tmp3, in0=g3[:, :, :C],
                    in1=mtile[:, :, None].to_broadcast([128, NL, C]),
                    op=OP.mult)
                nc.vector.tensor_tensor(out=thu3, in0=thu3, in1=tmp3, op=OP.add)

            def plain_gather(slot_d_, tag):
                gtile = tp.tile([128, NL * EPAD], bf16, tag=f"zg{tag}")
                itile = tp.tile([128, BPC // 16], i16, tag=f"zi{tag}")
                nc.sync.dma_start(out=itile[:], in_=slot_d_[:])
                nc.gpsimd.dma_gather(
                    out_ap=gtile[:].rearrange("p (s e) -> p s e", e=EPAD),
                    in_ap=z1pad_d[:],
                    idxs_ap=itile[:], num_idxs=BPC, num_idxs_reg=BPC,
                    elem_size=EPAD, single_packet=False, queue_num=next_q())
                return gtile[:].rearrange("p (s e) -> p s e", e=EPAD)

            z1p3 = plain_gather(pslot_d, "p")
            z1n3 = plain_gather(nslot_d, "n")

            g_pos = rowdot(thu3, z1p3[:, :, :C], C, "gp", odt=f32)
            g_neg = rowdot(thu3, z1n3[:, :, :C], C, "gn", odt=f32)

            mf1 = tp.tile([128, NL], f32, tag="mf1")
            nc.vector.tensor_tensor(out=mf1[:], in0=g_pos[:], in1=lsp[:],
                                    op=OP.mult)
            mf2 = tp.tile([128, NL], f32, tag="mf2")
            nc.vector.tensor_tensor(out=mf2[:], in0=g_neg[:], in1=lsn[:],
                                    op=OP.mult)
            nc.vector.tensor_tensor(out=mf1[:], in0=mf1[:], in1=mf2[:], op=OP.add)
            nc.vector.tensor_scalar(out=mf1[:], in0=mf1[:], scalar1=-1.0,
                                    scalar2=None, op0=OP.mult)
            nc.vector.reduce_sum(cols_t[:, 3:4], mf1[:], axis=AX.X)

            nc.vector.reduce_sum(cols_t[:, 4:5], g_pos[:], axis=AX.X)
            nc.vector.reduce_sum(cols_t[:, 5:6], g_neg[:], axis=AX.X)

            gu_acc = tp.tile([128, NL], f32, tag="gu")
            nc.vector.memset(gu_acc[:], 0.0)
            for gg in (g_pos, g_neg):
                lg = tp.tile([128, NL], f32, tag="lg")
                nc.scalar.activation(lg[:], gg[:], AF.Ln)
                nc.vector.tensor_scalar(out=lg[:], in0=lg[:], scalar1=-100.0,
                                        scalar2=None, op0=OP.max)
                omg = tp.tile([128, NL], f32, tag="omg")
                nc.vector.tensor_scalar(out=omg[:], in0=gg[:], scalar1=-1.0,
                                        scalar2=1.0, op0=OP.mult, op1=OP.add)
                l1g = tp.tile([128, NL], f32, tag="l1g")
                nc.scalar.activation(l1g[:], omg[:], AF.Ln)
                nc.vector.tensor_scalar(out=l1g[:], in0=l1g[:], scalar1=-100.0,
                                        scalar2=None, op0=OP.max)
                t1 = tp.tile([128, NL], f32, tag="gu_t1")
                nc.vector.tensor_tensor(out=t1[:], in0=gg[:], in1=lg[:], op=OP.mult)
                t2 = tp.tile([128, NL], f32, tag="gu_t2")
                nc.vector.tensor_tensor(out=t2[:], in0=omg[:], in1=l1g[:],
                                        op=OP.mult)
                nc.vector.tensor_tensor(out=t1[:], in0=t1[:], in1=t2[:], op=OP.add)
                nc.vector.tensor_tensor(out=gu_acc[:], in0=gu_acc[:], in1=t1[:],
                                        op=OP.add)
            nc.vector.tensor_scalar(out=gu_acc[:], in0=gu_acc[:], scalar1=-1.0,
                                    scalar2=None, op0=OP.mult)
            nc.vector.reduce_sum(cols_t[:, 6:7], gu_acc[:], axis=AX.X)

            with tc.tile_pool(name="fps", bufs=1, space="PSUM") as fp:
                pout = fp.tile([1, 16], f32, space="PSUM")
                nc.tensor.matmul(out=pout[:], lhsT=ones_t[:], rhs=cols_t[:],
                                 start=True, stop=True)
                pres = tp.tile([1, 16], f32, tag="pres")
                nc.vector.tensor_copy(out=pres[:], in_=pout[:])
                nc.sync.dma_start(out=parts_d[:], in_=pres[:])

    nc.compile()

    # Align each gather's SWDGE queue with the DMASW semaphore lane Tile
    # assigned it (lane k -> queue k % NQ). Tile's round-robin lane assignment
    # ignores queue_num, and ucode locks each completion sem to one queue.
    import re
    for f in nc.m.functions:
        for blk in f.blocks:
            for inst in blk.instructions:
                if isinstance(inst, mybir.InstDMAGatherAnt):
                    for up in inst.sync_info.on_update:
                        m_ = re.match(r"DMASW(\d+)_", up.ant_name or "")
                        if m_:
                            inst.queue_num = int(m_.group(1)) % NQ

    _BUILD_CACHE[S] = nc
    return nc


def run(inputs, trace=False, trace_kwargs=None):
    from concourse.bass_utils import run_bass_kernel_spmd
    in_maps, S = _prep(inputs)
    nc = _build(S)
    kw = {}
    if trace:
        kw["trace"] = True
        if trace_kwargs:
            kw.update(trace_kwargs)
    res = run_bass_kernel_spmd(nc, in_maps, core_ids=list(range(NCORES)), **kw)
    parts = [res.results[k]["parts"] for k in range(NCORES)]
    loss = _combine(parts)
    return loss, res


def kernel(**inputs) -> np.ndarray:
    loss, _ = run(inputs)
    return np.asarray(loss, dtype=np.float32)


# revision 9
# speedup vs baseline: 1.9845x; 1.1740x over previous
"""Trainium2 Bass kernel for the FAWMF loss (gnn_message_passing).

Strategy (8 NeuronCores, SPMD, no collectives):
  - Only item-rows of z matter, and only for items referenced by the batch
    (all_theta[col]=0 for col>=U, z[:U] is discarded by the reference).
  - Each core takes 1/8 of the (users, pos, neg) batch plus exactly the edges
    feeding its batch's items (need-based edge sharding). Items are remapped to
    <=4096 local slots (32 blocks of 128), balanced by degree.
  - Device: softmax(theta_user) is computed on-chip and written to a padded
    bf16 [U, 128] DRAM table (256B rows). Edge cols are gathered from it with
    GPSIMD dma_gather (int16 indices, 4 range-buckets, 4 SWDGE queues), scaled
    by edge vals, and segment-summed into z blocks via one-hot bf16 matmuls
    accumulating in fp32 PSUM. z1 = sigmoid(w1*z + w2) via ACT with
    per-partition scale/bias, written to a bf16 DRAM table; batch rows of
    z1/theta/embeddings are gathered back and reduced to 7 partial sums.
  - Host combines the 8 partial-sum vectors into the scalar loss.
"""
import os
import sys
import numpy as np

for _p in ("/opt/trn_rl_repo", "/root/.axon_site", "/root/.axon_site/_ro/trn_rl_repo",
           "/root/.axon_site/_ro/pypackages"):
    if os.path.isdir(_p) and _p not in sys.path:
        sys.path.append(_p)

import ml_dtypes
import concourse.bacc as bacc
import concourse.bass as bass
import concourse.mybir as mybir
import concourse.tile as tile

f32 = mybir.dt.float32
bf16 = mybir.dt.bfloat16
i16 = mybir.dt.int16
AF = mybir.ActivationFunctionType
OP = mybir.AluOpType
AX = mybir.AxisListType
np_bf16 = ml_dtypes.bfloat16

U, I, C, D, B = 100000, 50000, 32, 64, 16384
NCORES = 8
BPC = B // NCORES
NL = BPC // 128             # batch lanes per partition (16)
NBLK = 32                   # 128-row z blocks per core
NUB = 4                     # user-id buckets (int16 gather)
UBSZ = 25000
NIB = 2                     # item-id buckets
IBSZ = 32768
EPAD = 128                  # padded bf16 table row width (256B)
GRP = 4                     # blocks per gather group
NGRP = NBLK // GRP
NTT = 15                    # full softmax tiles (128 part x 50 rows)
QSM = 50                    # user rows per partition per softmax tile
WD = 1e-4
NQ = 4                      # SWDGE queues

# ----------------------------------------------------------------------------
# host-side sharding helpers
# ----------------------------------------------------------------------------

def _wrap16(idx):
    n = idx.shape[0]
    a = idx.reshape(n // 16, 16).T.astype(np.int16)
    return np.ascontiguousarray(np.tile(a, (8, 1)))


def _lanes(a, ncol):
    return np.ascontiguousarray(a.reshape(ncol, 128).T)


def _prep(inputs):
    users = np.asarray(inputs["users"]).astype(np.int64)
    pos = np.asarray(inputs["positive_items"]).astype(np.int64)
    neg = np.asarray(inputs["negative_items"]).astype(np.int64)
    er_full = np.asarray(inputs["edge_rows"])
    ec_full = np.asarray(inputs["edge_cols"])
    ev_full = np.asarray(inputs["edge_vals"])
    w1 = np.asarray(inputs["w1"], np.float32).reshape(-1)
    w2 = np.asarray(inputs["w2"], np.float32).reshape(-1)
    theta = np.asarray(inputs["theta_user"], np.float32)
    uemb = np.asarray(inputs["user_embedding"], np.float32)
    iemb = np.asarray(inputs["item_embedding"], np.float32)

    m = (er_full >= U) & (ec_full < U)
    er = (er_full[m] - U).astype(np.int64)
    ec = ec_full[m].astype(np.int64)
    ev = np.asarray(ev_full[m], np.float32)
    order = np.argsort(er, kind="stable")
    er, ec, ev = er[order], ec[order], ev[order]
    counts = np.bincount(er, minlength=I)
    ptr = np.zeros(I + 1, np.int64)
    np.cumsum(counts, out=ptr[1:])

    import heapq
    cores = []
    S_needed = 1
    for k in range(NCORES):
        u_k = users[k * BPC:(k + 1) * BPC]
        p_k = pos[k * BPC:(k + 1) * BPC]
        n_k = neg[k * BPC:(k + 1) * BPC]
        ids = np.unique(np.concatenate([p_k, n_k]))
        deg = counts[ids]
        o = np.argsort(-deg, kind="stable")
        blk_of = np.empty(len(ids), np.int64)
        slot_in_blk = np.empty(len(ids), np.int64)
        heap = [(0, 0, b) for b in range(NBLK)]
        heapq.heapify(heap)
        for i_ in o:
            while True:
                load, cnt, b = heapq.heappop(heap)
                if cnt < 128:
                    break
            blk_of[i_] = b
            slot_in_blk[i_] = cnt
            heapq.heappush(heap, (load + int(deg[i_]), cnt + 1, b))
        slot = blk_of * 128 + slot_in_blk
        slot_of_item = np.full(I, -1, np.int64)
        slot_of_item[ids] = slot

        lens = counts[ids]
        tot = int(lens.sum())
        startp = ptr[ids]
        idx = np.repeat(startp + lens - np.cumsum(lens), lens) + np.arange(tot)
        e_col = ec[idx]
        e_val = ev[idx]
        e_slot = np.repeat(slot, lens)
        e_lo = (e_slot & 127).astype(np.float32)
        e_bkt = e_col // UBSZ
        seg_key = (e_slot >> 7) * NUB + e_bkt
        seg_cnt = np.bincount(seg_key, minlength=NBLK * NUB)
        S_needed = max(S_needed, int(np.ceil(seg_cnt.max() / 128)))
        cores.append(dict(u_k=u_k, p_k=p_k, n_k=n_k, ids=ids, slot=slot,
                          slot_of_item=slot_of_item, e_col=e_col, e_val=e_val,
                          e_lo=e_lo, seg_key=seg_key))

    S = S_needed
    SEGE = S * 128
    TOTCH = NBLK * NUB * S
    TOTE = TOTCH * 128
    NI = GRP * SEGE

    seg_order = [(g * GRP + b_loc, r)
                 for g in range(NGRP) for r in range(NUB) for b_loc in range(GRP)]

    in_maps = []
    for k in range(NCORES):
        c = cores[k]
        segs_cols = np.zeros(TOTE, np.int64)
        segs_vals = np.zeros(TOTE, np.float32)
        segs_lo = np.zeros(TOTE, np.float32)
        sort_by_seg = np.argsort(c["seg_key"], kind="stable")
        seg_sorted_cols = c["e_col"][sort_by_seg]
        seg_sorted_vals = c["e_val"][sort_by_seg]
        seg_sorted_lo = c["e_lo"][sort_by_seg]
        seg_cnt = np.bincount(c["seg_key"], minlength=NBLK * NUB)
        seg_ptr = np.zeros(NBLK * NUB + 1, np.int64)
        np.cumsum(seg_cnt, out=seg_ptr[1:])
        off = 0
        for (b, r) in seg_order:
            sk = b * NUB + r
            ns = int(seg_cnt[sk])
            sl = slice(seg_ptr[sk], seg_ptr[sk] + ns)
            segs_cols[off:off + ns] = seg_sorted_cols[sl]
            segs_vals[off:off + ns] = seg_sorted_vals[sl]
            segs_lo[off:off + ns] = seg_sorted_lo[sl]
            segs_cols[off + ns:off + SEGE] = r * UBSZ
            off += SEGE

        cols16_parts = []
        for gi in range(NGRP * NUB):
            r = gi % NUB
            rel = segs_cols[gi * NI:(gi + 1) * NI] - r * UBSZ
            cols16_parts.append(_wrap16(rel.astype(np.int16)))
        cols16 = np.concatenate(cols16_parts, axis=1)

        lo_t = _lanes(segs_lo, TOTCH).astype(np_bf16)
        val_t = _lanes(segs_vals, TOTCH).astype(np_bf16)

        som = c["slot_of_item"]
        p_slot = som[c["p_k"]]
        n_slot = som[c["n_k"]]

        def bucketize(idx_global, nb, bsz):
            arrs, masks = [], []
            for r in range(nb):
                inb = (idx_global >= r * bsz) & (idx_global < (r + 1) * bsz)
                rel = np.where(inb, idx_global - r * bsz, 0)
                arrs.append(_wrap16(rel.astype(np.int16)))
                masks.append(_lanes(inb.astype(np.float32), NL))
            return arrs, masks

        uidx, umask = bucketize(c["u_k"], NUB, UBSZ)
        pidx, pmask = bucketize(c["p_k"], NIB, IBSZ)
        nidx, nmask = bucketize(c["n_k"], NIB, IBSZ)

        w1_loc = np.zeros((128, NBLK), np.float32)
        w2_loc = np.zeros((128, NBLK), np.float32)
        slot = c["slot"]
        w1_loc[slot & 127, slot >> 7] = w1[c["ids"]]
        w2_loc[slot & 127, slot >> 7] = w2[c["ids"]]

        th_sl = np.ascontiguousarray(
            theta[k * (U // NCORES):(k + 1) * (U // NCORES)].reshape(128, -1))
        wcat = np.concatenate([w1[k * (I // NCORES):(k + 1) * (I // NCORES)],
                               w2[k * (I // NCORES):(k + 1) * (I // NCORES)]])
        wpad = np.zeros(128 * 98, np.float32)
        wpad[:len(wcat)] = wcat
        w_sq = np.ascontiguousarray(wpad.reshape(98, 128).T)

        in_map = {
            "theta": theta, "uemb": uemb, "iemb": iemb,
            "cols16": cols16.astype(np.int16),
            "lo_t": lo_t, "val_t": val_t,
            "w1_loc": w1_loc, "w2_loc": w2_loc,
            "th_sq": th_sl.astype(np.float32), "w_sq": w_sq.astype(np.float32),
            "pslot": _wrap16(p_slot.astype(np.int16)),
            "nslot": _wrap16(n_slot.astype(np.int16)),
        }
        for r in range(NUB):
            in_map[f"uidx{r}"] = uidx[r]
            in_map[f"umask{r}"] = umask[r]
        for r in range(NIB):
            in_map[f"pidx{r}"] = pidx[r]
            in_map[f"pmask{r}"] = pmask[r]
            in_map[f"nidx{r}"] = nidx[r]
            in_map[f"nmask{r}"] = nmask[r]
        in_maps.append(in_map)

    return in_maps, S


def _combine(parts_list):
    p = np.sum(np.stack([np.asarray(x, np.float64).reshape(-1) for x in parts_list]),
               axis=0)
    th_sq, w_sq, reg1s, mf, sgp, sgn, gu = p[0], p[1], p[2], p[3], p[4], p[5], p[6]
    UA = -float(np.log(1e-3))
    UB_ = -float(np.log(1.0 - 1e-3))
    mean_mf = mf / (2 * B)
    mean_unknown = (B * (UA + UB_) - (sgp * UA + sgn * UB_)) / (2 * B)
    mean_gu = gu / (2 * B)
    loss = (mean_mf + 0.1 * (mean_unknown - mean_gu)
            + WD * (0.5 * reg1s / B + 0.5 * th_sq / U) + 0.1 * 0.5 * w_sq / I)
    return np.float32(loss)


# ----------------------------------------------------------------------------
# device program
# ----------------------------------------------------------------------------

_BUILD_CACHE = {}


def _build(S):
    if S in _BUILD_CACHE:
        return _BUILD_CACHE[S]
    SEGE = S * 128
    TOTCH = NBLK * NUB * S
    TOTE = TOTCH * 128
    NI = GRP * SEGE

    nc = bacc.Bacc("TRN2", target_bir_lowering=False, num_devices=NCORES,
                   num_swdge_queues=NQ)

    theta_d = nc.dram_tensor("theta", [U, C], f32, kind="ExternalInput")
    uemb_d = nc.dram_tensor("uemb", [U, D], f32, kind="ExternalInput")
    iemb_d = nc.dram_tensor("iemb", [I, D], f32, kind="ExternalInput")
    cols16_d = nc.dram_tensor("cols16", [128, TOTE // 16], i16, kind="ExternalInput")
    lo_d = nc.dram_tensor("lo_t", [128, TOTCH], bf16, kind="ExternalInput")
    val_d = nc.dram_tensor("val_t", [128, TOTCH], bf16, kind="ExternalInput")
    w1l_d = nc.dram_tensor("w1_loc", [128, NBLK], f32, kind="ExternalInput")
    w2l_d = nc.dram_tensor("w2_loc", [128, NBLK], f32, kind="ExternalInput")
    thsq_d = nc.dram_tensor("th_sq", [128, (U // NCORES) * C // 128], f32,
                            kind="ExternalInput")
    wsq_d = nc.dram_tensor("w_sq", [128, 98], f32, kind="ExternalInput")
    uidx_d = [nc.dram_tensor(f"uidx{r}", [128, BPC // 16], i16, kind="ExternalInput")
              for r in range(NUB)]
    umask_d = [nc.dram_tensor(f"umask{r}", [128, NL], f32, kind="ExternalInput")
               for r in range(NUB)]
    pidx_d = [nc.dram_tensor(f"pidx{r}", [128, BPC // 16], i16, kind="ExternalInput")
              for r in range(NIB)]
    pmask_d = [nc.dram_tensor(f"pmask{r}", [128, NL], f32, kind="ExternalInput")
               for r in range(NIB)]
    nidx_d = [nc.dram_tensor(f"nidx{r}", [128, BPC // 16], i16, kind="ExternalInput")
              for r in range(NIB)]
    nmask_d = [nc.dram_tensor(f"nmask{r}", [128, NL], f32, kind="ExternalInput")
               for r in range(NIB)]
    pslot_d = nc.dram_tensor("pslot", [128, BPC // 16], i16, kind="ExternalInput")
    nslot_d = nc.dram_tensor("nslot", [128, BPC // 16], i16, kind="ExternalInput")
    parts_d = nc.dram_tensor("parts", [1, 16], f32, kind="ExternalOutput")

    tpad_d = nc.dram_tensor("tpad", [U, EPAD], bf16)
    z1pad_d = nc.dram_tensor("z1pad", [NBLK * 128, EPAD], bf16)

    qrr = [0]

    def next_q():
        q = qrr[0]
        qrr[0] = (q + 1) % NQ
        return q

    with tile.TileContext(nc) as tc:
        with tc.tile_pool(name="const", bufs=1) as cpool, \
             tc.tile_pool(name="tail", bufs=1) as tp:
            iota_t = cpool.tile([128, 128], bf16)
            nc.gpsimd.iota(iota_t[:], pattern=[[1, 128]], base=0,
                           channel_multiplier=0,
                           allow_small_or_imprecise_dtypes=True)
            ones_t = cpool.tile([128, 1], f32)
            nc.vector.memset(ones_t[:], 1.0)
            cols_t = cpool.tile([128, 16], f32)
            nc.vector.memset(cols_t[:], 0.0)
            iota_rep = None  # set after S known tiles allocated below

            # ---- batch embedding gathers (independent of softmax) ----
            ue = tp.tile([128, NL * D], f32, tag="ue")
            pe = tp.tile([128, NL * D], f32, tag="pe")
            ne = tp.tile([128, NL * D], f32, tag="ne")
            for t_ in (ue, pe, ne):
                nc.vector.memset(t_[:], 0.0)
            ue3 = ue[:].rearrange("p (s d) -> p s d", d=D)
            pe3 = pe[:].rearrange("p (s d) -> p s d", d=D)
            ne3 = ne[:].rearrange("p (s d) -> p s d", d=D)

            _mg = [0]

            def masked_gather_f32(acc3, table_view, idx_d_, mask_d_):
                u = _mg[0]; _mg[0] += 1
                gtile = tp.tile([128, NL * D], f32, tag=f"bg{u}")
                itile = tp.tile([128, BPC // 16], i16, tag=f"bidx{u}")
                mtile = tp.tile([128, NL], f32, tag=f"bmask{u}")
                nc.sync.dma_start(out=itile[:], in_=idx_d_[:])
                nc.sync.dma_start(out=mtile[:], in_=mask_d_[:])
                nc.gpsimd.dma_gather(
                    out_ap=gtile[:].rearrange("p (s e) -> p s e", e=D),
                    in_ap=table_view,
                    idxs_ap=itile[:], num_idxs=BPC, num_idxs_reg=BPC,
                    elem_size=D, single_packet=False, queue_num=next_q())
                g3 = gtile[:].rearrange("p (s e) -> p s e", e=D)
                tmp = tp.tile([128, NL * D], f32, tag=f"btmp{u}")
                tmp3 = tmp[:].rearrange("p (s e) -> p s e", e=D)
                nc.vector.tensor_tensor(
                    out=tmp3, in0=g3,
                    in1=mtile[:, :, None].to_broadcast([128, NL, D]),
                    op=OP.mult)
                nc.vector.tensor_tensor(out=acc3, in0=acc3, in1=tmp3, op=OP.add)

            for r in range(NUB):
                masked_gather_f32(ue3, uemb_d[r * UBSZ:(r + 1) * UBSZ, :],
                                  uidx_d[r], umask_d[r])
            for r in range(NIB):
                sz = min(IBSZ, I - r * IBSZ)
                masked_gather_f32(pe3, iemb_d[r * IBSZ:r * IBSZ + sz, :],
                                  pidx_d[r], pmask_d[r])
                masked_gather_f32(ne3, iemb_d[r * IBSZ:r * IBSZ + sz, :],
                                  nidx_d[r], nmask_d[r])

            # scores + bce + reg1 (independent of softmax table)
            def rowdot(in0_3, in1_3, width, tag, odt=f32):
                prod = tp.tile([128, NL * width], odt, tag=f"{tag}_p")
                p3 = prod[:].rearrange("p (s e) -> p s e", e=width)
                nc.vector.tensor_tensor(out=p3, in0=in0_3, in1=in1_3, op=OP.mult)
                out = tp.tile([128, NL], f32, tag=f"{tag}_r")
                nc.vector.reduce_sum(out[:], p3, axis=AX.X)
                return out

            s_pos = rowdot(ue3, pe3, D, "sp")
            s_neg = rowdot(ue3, ne3, D, "sn")

            sigp = tp.tile([128, NL], f32, tag="sigp")
            nc.scalar.activation(sigp[:], s_pos[:], AF.Sigmoid)
            lsp = tp.tile([128, NL], f32, tag="lsp")
            nc.scalar.activation(lsp[:], sigp[:], AF.Ln)
            nc.vector.tensor_scalar(out=lsp[:], in0=lsp[:], scalar1=-100.0,
                                    scalar2=None, op0=OP.max)
            sign_ = tp.tile([128, NL], f32, tag="sign")
            nc.scalar.activation(sign_[:], s_neg[:], AF.Sigmoid, scale=-1.0)
            lsn = tp.tile([128, NL], f32, tag="lsn")
            nc.scalar.activation(lsn[:], sign_[:], AF.Ln)
            nc.vector.tensor_scalar(out=lsn[:], in0=lsn[:], scalar1=-100.0,
                                    scalar2=None, op0=OP.max)

            r1 = tp.tile([128, 1], f32, tag="r1")
            nc.vector.memset(r1[:], 0.0)
            for emb in (ue, pe, ne):
                sq = tp.tile([128, NL * D], f32, tag="r1sq")
                nc.vector.tensor_tensor(out=sq[:], in0=emb[:], in1=emb[:],
                                        op=OP.mult)
                rs = tp.tile([128, 1], f32, tag="r1rs")
                nc.vector.reduce_sum(rs[:], sq[:], axis=AX.X)
                nc.vector.tensor_tensor(out=r1[:], in0=r1[:], in1=rs[:], op=OP.add)
            nc.vector.tensor_copy(out=cols_t[:, 2:3], in_=r1[:])

            # ---------------- phase A: softmax -> padded bf16 DRAM table -------
            with tc.tile_pool(name="sm", bufs=2) as sm:
                for t in range(NTT + 1):
                    P = 128 if t < NTT else (U - NTT * 128 * QSM) // QSM
                    rows = slice(t * 128 * QSM, t * 128 * QSM + P * QSM)
                    tin = sm.tile([128, QSM * C], f32, tag="tin")
                    nc.sync.dma_start(
                        out=tin[:P],
                        in_=theta_d[rows, :].rearrange("(p q) c -> p (q c)", q=QSM))
                    te = sm.tile([128, QSM * C], f32, tag="te")
                    nc.scalar.activation(te[:P], tin[:P], AF.Exp)
                    te3 = te[:P].rearrange("p (q c) -> p q c", c=C)
                    ts = sm.tile([128, QSM], f32, tag="ts")
                    nc.vector.reduce_sum(ts[:P], te3, axis=AX.X)
                    tr = sm.tile([128, QSM], f32, tag="tr")
                    nc.vector.reciprocal(tr[:P], ts[:P])
                    tout = sm.tile([128, QSM * EPAD], bf16, tag="tout")
                    t3 = tout[:P].rearrange("p (q e) -> p q e", e=EPAD)
                    nc.scalar.memzero(t3[:, :, C:])
                    nc.vector.tensor_tensor(
                        out=t3[:, :, :C], in0=te3,
                        in1=tr[:P, :, None].to_broadcast([P, QSM, C]),
                        op=OP.mult)
                    nc.sync.dma_start(
                        out=tpad_d[rows, :].rearrange("(p q) e -> p (q e)", q=QSM),
                        in_=tout[:P])

            # ---- reg partials ----
            with tc.tile_pool(name="regs", bufs=1) as rp:
                thsq_t = rp.tile([128, (U // NCORES) * C // 128], f32, tag="thsq")
                nc.sync.dma_start(out=thsq_t[:], in_=thsq_d[:])
                thsq2 = rp.tile([128, (U // NCORES) * C // 128], f32, tag="thsq2")
                nc.vector.tensor_tensor(out=thsq2[:], in0=thsq_t[:], in1=thsq_t[:],
                                        op=OP.mult)
                nc.vector.reduce_sum(cols_t[:, 0:1], thsq2[:], axis=AX.X)
                wsq_t = rp.tile([128, 98], f32, tag="wsq")
                nc.sync.dma_start(out=wsq_t[:], in_=wsq_d[:])
                wsq2 = rp.tile([128, 98], f32, tag="wsq2")
                nc.vector.tensor_tensor(out=wsq2[:], in0=wsq_t[:], in1=wsq_t[:],
                                        op=OP.mult)
                nc.vector.reduce_sum(cols_t[:, 1:2], wsq2[:], axis=AX.X)

            # ---------------- phase C: edges -> z -> z1pad ---------------------
            w1l_t = cpool.tile([128, NBLK], f32)
            w2l_t = cpool.tile([128, NBLK], f32)
            nc.sync.dma_start(out=w1l_t[:], in_=w1l_d[:])
            nc.sync.dma_start(out=w2l_t[:], in_=w2l_d[:])
            lo_full = cpool.tile([128, TOTCH], bf16)
            val_full = cpool.tile([128, TOTCH], bf16)
            nc.sync.dma_start(out=lo_full[:], in_=lo_d[:])
            nc.sync.dma_start(out=val_full[:], in_=val_d[:])
            c16_full = cpool.tile([128, TOTE // 16], i16)
            nc.sync.dma_start(out=c16_full[:], in_=cols16_d[:])
            iota_rep = cpool.tile([128, S * 128], bf16)
            nc.vector.tensor_copy(
                out=iota_rep[:].rearrange("p (s q) -> p s q", q=128),
                in_=iota_t[:, None, :].to_broadcast([128, S, 128]))

            with tc.tile_pool(name="main", bufs=1) as mp, \
                 tc.tile_pool(name="mm", bufs=2) as mm, \
                 tc.tile_pool(name="psum", bufs=2, space="PSUM") as pp:
                for g in range(NGRP):
                    gts = []
                    for r in range(NUB):
                        gi = g * NUB + r
                        gt = mp.tile([128, GRP * S * EPAD], bf16, tag=f"G{r}")
                        nc.gpsimd.dma_gather(
                            out_ap=gt[:].rearrange("p (s e) -> p s e", e=EPAD),
                            in_ap=tpad_d[r * UBSZ:(r + 1) * UBSZ, :],
                            idxs_ap=c16_full[:, gi * (NI // 16):(gi + 1) * (NI // 16)],
                            num_idxs=NI, num_idxs_reg=NI, elem_size=EPAD,
                            single_packet=False, queue_num=r)
                        gt3 = gt[:].rearrange("p (s e) -> p s e", e=EPAD)
                        cbase = gi * GRP * S
                        nc.vector.tensor_tensor(
                            out=gt3[:, :, :C], in0=gt3[:, :, :C],
                            in1=val_full[:, cbase:cbase + GRP * S, None]
                                .to_broadcast([128, GRP * S, C]),
                            op=OP.mult)
                        gts.append(gt)
                    for b_loc in range(GRP):
                        b = g * GRP + b_loc
                        zp = pp.tile([128, C], f32, space="PSUM", tag="zp")
                        for r in range(NUB):
                            cbase = (g * NUB + r) * GRP * S
                            mt = mm.tile([128, S * 128], bf16, tag="M")
                            nc.vector.tensor_tensor(
                                out=mt[:].rearrange("p (s q) -> p s q", q=128),
                                in0=iota_rep[:].rearrange("p (s q) -> p s q", q=128),
                                in1=lo_full[:, cbase + b_loc * S:
                                            cbase + (b_loc + 1) * S, None]
                                    .to_broadcast([128, S, 128]),
                                op=OP.is_equal)
                            g3 = gts[r][:].rearrange("p (s e) -> p s e", e=EPAD)
                            for s in range(S):
                                nc.tensor.matmul(
                                    out=zp[:],
                                    lhsT=mt[:, s * 128:(s + 1) * 128],
                                    rhs=g3[:, b_loc * S + s, :C],
                                    start=(r == 0 and s == 0),
                                    stop=(r == NUB - 1 and s == S - 1))
                        z1s = mm.tile([128, EPAD], bf16, tag="z1s")
                        nc.vector.memset(z1s[:, C:], 0.0)
                        nc.scalar.activation(z1s[:, :C], zp[:], AF.Sigmoid,
                                             bias=w2l_t[:, b:b + 1],
                                             scale=w1l_t[:, b:b + 1])
                        nc.sync.dma_start(out=z1pad_d[b * 128:(b + 1) * 128, :],
                                          in_=z1s[:])

            # ---------------- phase D: softmax-dependent batch tail ------------
            thu = tp.tile([128, NL * C], bf16, tag="thu")
            nc.vector.memset(thu[:], 0.0)
            thu3 = thu[:].rearrange("p (s c) -> p s c", c=C)
            for r in range(NUB):
                gtile = tp.tile([128, NL * EPAD], bf16, tag="tg")
                itile = tp.tile([128, BPC // 16], i16, tag="tgidx")
                mtile = tp.tile([128, NL], bf16, tag="tgmask")
                mtile_f = tp.tile([128, NL], f32, tag="tgmaskf")
                nc.sync.dma_start(out=itile[:], in_=uidx_d[r][:])
                nc.sync.dma_start(out=mtile_f[:], in_=umask_d[r][:])
                nc.vector.tensor_copy(out=mtile[:], in_=mtile_f[:])
                nc.gpsimd.dma_gather(
                    out_ap=gtile[:].rearrange("p (s e) -> p s e", e=EPAD),
                    in_ap=tpad_d[r * UBSZ:(r + 1) * UBSZ, :],
                    idxs_ap=itile[:], num_idxs=BPC, num_idxs_reg=BPC,
                    elem_size=EPAD, single_packet=False, queue_num=next_q())
                g3 = gtile[:].rearrange("p (s e) -> p s e", e=EPAD)
                tmp = tp.tile([128, NL * C], bf16, tag="ttmp")
                tmp3 = tmp[:].rearrange("p (s e) -> p s e", e=C)
                nc.vector.tensor_tensor(
                    out=tmp3, in0=g3[:, :, :C],
                    in1=mtile[:, :, None].to_broadcast([128, NL, C]),
                    op=OP.mult)
                nc.vector.tensor_tensor(out=thu3, in0=thu3, in1=tmp3, op=OP.add)

            def plain_gather(slot_d_, tag):
                gtile = tp.tile([128, NL * EPAD], bf16, tag=f"zg{tag}")
                itile = tp.tile([128, BPC // 16], i16, tag=f"zi{tag}")
                nc.sync.dma_start(out=itile[:], in_=slot_d_[:])
                nc.gpsimd.dma_gather(
                    out_ap=gtile[:].rearrange("p (s e) -> p s e", e=EPAD),
                    in_ap=z1pad_d[:],
                    idxs_ap=itile[:], num_idxs=BPC, num_idxs_reg=BPC,
                    elem_size=EPAD, single_packet=False, queue_num=next_q())
                return gtile[:].rearrange("p (s e) -> p s e", e=EPAD)

            z1p3 = plain_gather(pslot_d, "p")
            z1n3 = plain_gather(nslot_d, "n")

            g_pos = rowdot(thu3, z1p3[:, :, :C], C, "gp", odt=f32)
            g_neg = rowdot(thu3, z1n3[:, :, :C], C, "gn", odt=f32)

            mf1 = tp.tile([128, NL], f32, tag="mf1")
            nc.vector.tensor_tensor(out=mf1[:], in0=g_pos[:], in1=lsp[:],
                                    op=OP.mult)
            mf2 = tp.tile([128, NL], f32, tag="mf2")
            nc.vector.tensor_tensor(out=mf2[:], in0=g_neg[:], in1=lsn[:],
                                    op=OP.mult)
            nc.vector.tensor_tensor(out=mf1[:], in0=mf1[:], in1=mf2[:], op=OP.add)
            nc.vector.tensor_scalar(out=mf1[:], in0=mf1[:], scalar1=-1.0,
                                    scalar2=None, op0=OP.mult)
            nc.vector.reduce_sum(cols_t[:, 3:4], mf1[:], axis=AX.X)

            nc.vector.reduce_sum(cols_t[:, 4:5], g_pos[:], axis=AX.X)
            nc.vector.reduce_sum(cols_t[:, 5:6], g_neg[:], axis=AX.X)

            gu_acc = tp.tile([128, NL], f32, tag="gu")
            nc.vector.memset(gu_acc[:], 0.0)
            for gg in (g_pos, g_neg):
                lg = tp.tile([128, NL], f32, tag="lg")
                nc.scalar.activation(lg[:], gg[:], AF.Ln)
                nc.vector.tensor_scalar(out=lg[:], in0=lg[:], scalar1=-100.0,
                                        scalar2=None, op0=OP.max)
                omg = tp.tile([128, NL], f32, tag="omg")
                nc.vector.tensor_scalar(out=omg[:], in0=gg[:], scalar1=-1.0,
                                        scalar2=1.0, op0=OP.mult, op1=OP.add)
                l1g = tp.tile([128, NL], f32, tag="l1g")
                nc.scalar.activation(l1g[:], omg[:], AF.Ln)
                nc.vector.tensor_scalar(out=l1g[:], in0=l1g[:], scalar1=-100.0,
                                        scalar2=None, op0=OP.max)
                t1 = tp.tile([128, NL], f32, tag="gu_t1")
                nc.vector.tensor_tensor(out=t1[:], in0=gg[:], in1=lg[:], op=OP.mult)
                t2 = tp.tile([128, NL], f32, tag="gu_t2")
                nc.vector.tensor_tensor(out=t2[:], in0=omg[:], in1=l1g[:],
                                        op=OP.mult)
                nc.vector.tensor_tensor(out=t1[:], in0=t1[:], in1=t2[:], op=OP.add)
                nc.vector.tensor_tensor(out=gu_acc[:], in0=gu_acc[:], in1=t1[:],
                                        op=OP.add)
            nc.vector.tensor_scalar(out=gu_acc[:], in0=gu_acc[:], scalar1=-1.0,
                                    scalar2=None, op0=OP.mult)
            nc.vector.reduce_sum(cols_t[:, 6:7], gu_acc[:], axis=AX.X)

            with tc.tile_pool(name="fps", bufs=1, space="PSUM") as fp:
                pout = fp.tile([1, 16], f32, space="PSUM")
                nc.tensor.matmul(out=pout[:], lhsT=ones_t[:], rhs=cols_t[:],
                                 start=True, stop=True)
                pres = tp.tile([1, 16], f32, tag="pres")
                nc.vector.tensor_copy(out=pres[:], in_=pout[:])
                nc.sync.dma_start(out=parts_d[:], in_=pres[:])

    nc.compile()

    # Align each gather's SWDGE queue with the DMASW semaphore lane Tile
    # assigned it (lane k -> queue k % NQ). Tile's round-robin lane assignment
    # ignores queue_num, and ucode locks each completion sem to one queue.
    import re
    for f in nc.m.functions:
        for blk in f.blocks:
            for inst in blk.instructions:
                if isinstance(inst, mybir.InstDMAGatherAnt):
                    for up in inst.sync_info.on_update:
                        m_ = re.match(r"DMASW(\d+)_", up.ant_name or "")
                        if m_:
                            inst.queue_num = int(m_.group(1)) % NQ

    _BUILD_CACHE[S] = nc
    return nc


def run(inputs, trace=False, trace_kwargs=None):
    from concourse.bass_utils import run_bass_kernel_spmd
    in_maps, S = _prep(inputs)
    nc = _build(S)
    kw = {}
    if trace:
        kw["trace"] = True
        if trace_kwargs:
            kw.update(trace_kwargs)
    res = run_bass_kernel_spmd(nc, in_maps, core_ids=list(range(NCORES)), **kw)
    parts = [res.results[k]["parts"] for k in range(NCORES)]
    loss = _combine(parts)
    return loss, res


def kernel(**inputs) -> np.ndarray:
    loss, _ = run(inputs)
    return np.asarray(loss, dtype=np.float32)


# revision 11
# speedup vs baseline: 2.7328x; 1.3771x over previous
"""Trainium2 Bass kernel for the FAWMF loss (gnn_message_passing).

Strategy (8 NeuronCores, SPMD, no collectives):
  - Only item-rows of z matter, and only for items referenced by the batch
    (all_theta[col]=0 for col>=U, z[:U] is discarded by the reference).
  - Each core takes 1/8 of the (users, pos, neg) batch plus exactly the edges
    feeding its batch's items (need-based edge sharding). Items are remapped to
    <=4096 local slots (32 blocks of 128), balanced by degree.
  - Device: softmax(theta_user) is computed on-chip and written to a padded
    bf16 [U, 128] DRAM table (256B rows). Edge cols are gathered from it with
    GPSIMD dma_gather (int16 indices, 4 range-buckets, 4 SWDGE queues), scaled
    by edge vals, and segment-summed into z blocks via one-hot bf16 matmuls
    accumulating in fp32 PSUM. z1 = sigmoid(w1*z + w2) via ACT with
    per-partition scale/bias, written to a bf16 DRAM table; batch rows of
    z1/theta/embeddings are gathered back and reduced to 7 partial sums.
  - Host combines the 8 partial-sum vectors into the scalar loss.
"""
import os
import sys
import numpy as np

for _p in ("/opt/trn_rl_repo", "/root/.axon_site", "/root/.axon_site/_ro/trn_rl_repo",
           "/root/.axon_site/_ro/pypackages"):
    if os.path.isdir(_p) and _p not in sys.path:
        sys.path.append(_p)

import ml_dtypes
import concourse.bacc as bacc
import concourse.bass as bass
import concourse.mybir as mybir
import concourse.tile as tile

f32 = mybir.dt.float32
bf16 = mybir.dt.bfloat16
i16 = mybir.dt.int16
AF = mybir.ActivationFunctionType
OP = mybir.AluOpType
AX = mybir.AxisListType
np_bf16 = ml_dtypes.bfloat16

U, I, C, D, B = 100000, 50000, 32, 64, 16384
NCORES = 8
BPC = B // NCORES
NL = BPC // 128             # batch lanes per partition (16)
NBLK = 32                   # 128-row z blocks per core
NUB = 4                     # user-id buckets (int16 gather)
UBSZ = 25000
NIB = 2                     # item-id buckets
IBSZ = 32768
EPAD = 128                  # padded bf16 table row width (256B)
GRP = 4                     # blocks per gather group
NGRP = NBLK // GRP
NTT = 15                    # full softmax tiles (128 part x 50 rows)
QSM = 50                    # user rows per partition per softmax tile
WD = 1e-4
NQ = 4                      # SWDGE queues

# ----------------------------------------------------------------------------
# host-side sharding helpers
# ----------------------------------------------------------------------------

def _wrap16(idx):
    n = idx.shape[0]
    a = idx.reshape(n // 16, 16).T.astype(np.int16)
    return np.ascontiguousarray(np.tile(a, (8, 1)))


def _lanes(a, ncol):
    return np.ascontiguousarray(a.reshape(ncol, 128).T)


def _prep(inputs):
    users = np.asarray(inputs["users"]).astype(np.int64)
    pos = np.asarray(inputs["positive_items"]).astype(np.int64)
    neg = np.asarray(inputs["negative_items"]).astype(np.int64)
    er_full = np.asarray(inputs["edge_rows"])
    ec_full = np.asarray(inputs["edge_cols"])
    ev_full = np.asarray(inputs["edge_vals"])
    w1 = np.asarray(inputs["w1"], np.float32).reshape(-1)
    w2 = np.asarray(inputs["w2"], np.float32).reshape(-1)
    theta = np.asarray(inputs["theta_user"], np.float32)
    uemb = np.asarray(inputs["user_embedding"], np.float32)
    iemb = np.asarray(inputs["item_embedding"], np.float32)

    m = (er_full >= U) & (ec_full < U)
    er = (er_full[m] - U).astype(np.int64)
    ec = ec_full[m].astype(np.int64)
    ev = np.asarray(ev_full[m], np.float32)
    order = np.argsort(er, kind="stable")
    er, ec, ev = er[order], ec[order], ev[order]
    counts = np.bincount(er, minlength=I)
    ptr = np.zeros(I + 1, np.int64)
    np.cumsum(counts, out=ptr[1:])

    import heapq
    cores = []
    S_needed = 1
    for k in range(NCORES):
        u_k = users[k * BPC:(k + 1) * BPC]
        p_k = pos[k * BPC:(k + 1) * BPC]
        n_k = neg[k * BPC:(k + 1) * BPC]
        ids = np.unique(np.concatenate([p_k, n_k]))
        deg = counts[ids]
        o = np.argsort(-deg, kind="stable")
        blk_of = np.empty(len(ids), np.int64)
        slot_in_blk = np.empty(len(ids), np.int64)
        heap = [(0, 0, b) for b in range(NBLK)]
        heapq.heapify(heap)
        for i_ in o:
            while True:
                load, cnt, b = heapq.heappop(heap)
                if cnt < 128:
                    break
            blk_of[i_] = b
            slot_in_blk[i_] = cnt
            heapq.heappush(heap, (load + int(deg[i_]), cnt + 1, b))
        slot = blk_of * 128 + slot_in_blk
        slot_of_item = np.full(I, -1, np.int64)
        slot_of_item[ids] = slot

        lens = counts[ids]
        tot = int(lens.sum())
        startp = ptr[ids]
        idx = np.repeat(startp + lens - np.cumsum(lens), lens) + np.arange(tot)
        e_col = ec[idx]
        e_val = ev[idx]
        e_slot = np.repeat(slot, lens)
        e_lo = (e_slot & 127).astype(np.float32)
        e_bkt = e_col // UBSZ
        seg_key = (e_slot >> 7) * NUB + e_bkt
        seg_cnt = np.bincount(seg_key, minlength=NBLK * NUB)
        S_needed = max(S_needed, int(np.ceil(seg_cnt.max() / 128)))
        cores.append(dict(u_k=u_k, p_k=p_k, n_k=n_k, ids=ids, slot=slot,
                          slot_of_item=slot_of_item, e_col=e_col, e_val=e_val,
                          e_lo=e_lo, seg_key=seg_key))

    S = S_needed
    SEGE = S * 128
    TOTCH = NBLK * NUB * S
    TOTE = TOTCH * 128
    NI = GRP * SEGE

    seg_order = [(g * GRP + b_loc, r)
                 for g in range(NGRP) for r in range(NUB) for b_loc in range(GRP)]

    in_maps = []
    for k in range(NCORES):
        c = cores[k]
        segs_cols = np.zeros(TOTE, np.int64)
        segs_vals = np.zeros(TOTE, np.float32)
        segs_lo = np.zeros(TOTE, np.float32)
        sort_by_seg = np.argsort(c["seg_key"], kind="stable")
        seg_sorted_cols = c["e_col"][sort_by_seg]
        seg_sorted_vals = c["e_val"][sort_by_seg]
        seg_sorted_lo = c["e_lo"][sort_by_seg]
        seg_cnt = np.bincount(c["seg_key"], minlength=NBLK * NUB)
        seg_ptr = np.zeros(NBLK * NUB + 1, np.int64)
        np.cumsum(seg_cnt, out=seg_ptr[1:])
        off = 0
        for (b, r) in seg_order:
            sk = b * NUB + r
            ns = int(seg_cnt[sk])
            sl = slice(seg_ptr[sk], seg_ptr[sk] + ns)
            segs_cols[off:off + ns] = seg_sorted_cols[sl]
            segs_vals[off:off + ns] = seg_sorted_vals[sl]
            segs_lo[off:off + ns] = seg_sorted_lo[sl]
            segs_cols[off + ns:off + SEGE] = r * UBSZ
            off += SEGE

        cols16_parts = []
        for gi in range(NGRP * NUB):
            r = gi % NUB
            rel = segs_cols[gi * NI:(gi + 1) * NI] - r * UBSZ
            cols16_parts.append(_wrap16(rel.astype(np.int16)))
        cols16 = np.concatenate(cols16_parts, axis=1)

        lo_t = _lanes(segs_lo, TOTCH).astype(np_bf16)
        val_t = _lanes(segs_vals, TOTCH).astype(np_bf16)

        som = c["slot_of_item"]
        p_slot = som[c["p_k"]]
        n_slot = som[c["n_k"]]

        def bucketize(idx_global, nb, bsz):
            arrs, masks = [], []
            for r in range(nb):
                inb = (idx_global >= r * bsz) & (idx_global < (r + 1) * bsz)
                rel = np.where(inb, idx_global - r * bsz, 0)
                arrs.append(_wrap16(rel.astype(np.int16)))
                masks.append(_lanes(inb.astype(np.float32), NL))
            return arrs, masks

        uidx, umask = bucketize(c["u_k"], NUB, UBSZ)
        pidx, pmask = bucketize(c["p_k"], NIB, IBSZ)
        nidx, nmask = bucketize(c["n_k"], NIB, IBSZ)

        w1_loc = np.zeros((128, NBLK), np.float32)
        w2_loc = np.zeros((128, NBLK), np.float32)
        slot = c["slot"]
        w1_loc[slot & 127, slot >> 7] = w1[c["ids"]]
        w2_loc[slot & 127, slot >> 7] = w2[c["ids"]]

        th_sl = np.ascontiguousarray(
            theta[k * (U // NCORES):(k + 1) * (U // NCORES)].reshape(128, -1))
        wcat = np.concatenate([w1[k * (I // NCORES):(k + 1) * (I // NCORES)],
                               w2[k * (I // NCORES):(k + 1) * (I // NCORES)]])
        wpad = np.zeros(128 * 98, np.float32)
        wpad[:len(wcat)] = wcat
        w_sq = np.ascontiguousarray(wpad.reshape(98, 128).T)

        in_map = {
            "theta": theta, "uemb": uemb, "iemb": iemb,
            "cols16": cols16.astype(np.int16),
            "lo_t": lo_t, "val_t": val_t,
            "w1_loc": w1_loc, "w2_loc": w2_loc,
            "th_sq": th_sl.astype(np.float32), "w_sq": w_sq.astype(np.float32),
            "pslot": _wrap16(p_slot.astype(np.int16)),
            "nslot": _wrap16(n_slot.astype(np.int16)),
        }
        for r in range(NUB):
            in_map[f"uidx{r}"] = uidx[r]
            in_map[f"umask{r}"] = umask[r]
        for r in range(NIB):
            in_map[f"pidx{r}"] = pidx[r]
            in_map[f"pmask{r}"] = pmask[r]
            in_map[f"nidx{r}"] = nidx[r]
            in_map[f"nmask{r}"] = nmask[r]
        in_maps.append(in_map)

    return in_maps, S


def _combine(parts_list):
    p = np.sum(np.stack([np.asarray(x, np.float64).reshape(-1) for x in parts_list]),
               axis=0)
    th_sq, w_sq, reg1s, mf, sgp, sgn, gu = p[0], p[1], p[2], p[3], p[4], p[5], p[6]
    UA = -float(np.log(1e-3))
    UB_ = -float(np.log(1.0 - 1e-3))
    mean_mf = mf / (2 * B)
    mean_unknown = (B * (UA + UB_) - (sgp * UA + sgn * UB_)) / (2 * B)
    mean_gu = gu / (2 * B)
    loss = (mean_mf + 0.1 * (mean_unknown - mean_gu)
            + WD * (0.5 * reg1s / B + 0.5 * th_sq / U) + 0.1 * 0.5 * w_sq / I)
    return np.float32(loss)


# ----------------------------------------------------------------------------
# device program
# ----------------------------------------------------------------------------

_BUILD_CACHE = {}


def _build(S):
    if S in _BUILD_CACHE:
        return _BUILD_CACHE[S]
    SEGE = S * 128
    TOTCH = NBLK * NUB * S
    TOTE = TOTCH * 128
    NI = GRP * SEGE

    nc = bacc.Bacc("TRN2", target_bir_lowering=False, num_devices=NCORES,
                   num_swdge_queues=NQ)

    theta_d = nc.dram_tensor("theta", [U, C], f32, kind="ExternalInput")
    uemb_d = nc.dram_tensor("uemb", [U, D], f32, kind="ExternalInput")
    iemb_d = nc.dram_tensor("iemb", [I, D], f32, kind="ExternalInput")
    cols16_d = nc.dram_tensor("cols16", [128, TOTE // 16], i16, kind="ExternalInput")
    lo_d = nc.dram_tensor("lo_t", [128, TOTCH], bf16, kind="ExternalInput")
    val_d = nc.dram_tensor("val_t", [128, TOTCH], bf16, kind="ExternalInput")
    w1l_d = nc.dram_tensor("w1_loc", [128, NBLK], f32, kind="ExternalInput")
    w2l_d = nc.dram_tensor("w2_loc", [128, NBLK], f32, kind="ExternalInput")
    thsq_d = nc.dram_tensor("th_sq", [128, (U // NCORES) * C // 128], f32,
                            kind="ExternalInput")
    wsq_d = nc.dram_tensor("w_sq", [128, 98], f32, kind="ExternalInput")
    uidx_d = [nc.dram_tensor(f"uidx{r}", [128, BPC // 16], i16, kind="ExternalInput")
              for r in range(NUB)]
    umask_d = [nc.dram_tensor(f"umask{r}", [128, NL], f32, kind="ExternalInput")
               for r in range(NUB)]
    pidx_d = [nc.dram_tensor(f"pidx{r}", [128, BPC // 16], i16, kind="ExternalInput")
              for r in range(NIB)]
    pmask_d = [nc.dram_tensor(f"pmask{r}", [128, NL], f32, kind="ExternalInput")
               for r in range(NIB)]
    nidx_d = [nc.dram_tensor(f"nidx{r}", [128, BPC // 16], i16, kind="ExternalInput")
              for r in range(NIB)]
    nmask_d = [nc.dram_tensor(f"nmask{r}", [128, NL], f32, kind="ExternalInput")
               for r in range(NIB)]
    pslot_d = nc.dram_tensor("pslot", [128, BPC // 16], i16, kind="ExternalInput")
    nslot_d = nc.dram_tensor("nslot", [128, BPC // 16], i16, kind="ExternalInput")
    parts_d = nc.dram_tensor("parts", [1, 16], f32, kind="ExternalOutput")

    tpad_d = nc.dram_tensor("tpad", [U, EPAD], bf16)
    z1pad_d = nc.dram_tensor("z1pad", [NBLK * 128, EPAD], bf16)

    qrr = [0]

    def next_q():
        q = qrr[0]
        qrr[0] = (q + 1) % NQ
        return q

    with tile.TileContext(nc) as tc:
        with tc.tile_pool(name="const", bufs=1) as cpool, \
             tc.tile_pool(name="tail", bufs=1) as tp:
            bsp_ctx = tc.tile_pool(name="bsp", bufs=4)
            bsp = bsp_ctx.__enter__()
            iota_t = cpool.tile([128, 128], bf16)
            nc.gpsimd.iota(iota_t[:], pattern=[[1, 128]], base=0,
                           channel_multiplier=0,
                           allow_small_or_imprecise_dtypes=True)
            ones_t = cpool.tile([128, 1], f32)
            nc.vector.memset(ones_t[:], 1.0)
            cols_t = cpool.tile([128, 16], f32)
            nc.vector.memset(cols_t[:], 0.0)
            iota_rep = None  # set after S known tiles allocated below

            # ---- batch embedding gathers (independent of softmax) ----
            ue = tp.tile([128, NL * D], f32, tag="ue")
            pe = tp.tile([128, NL * D], f32, tag="pe")
            ne = tp.tile([128, NL * D], f32, tag="ne")
            for t_ in (ue, pe, ne):
                nc.vector.memset(t_[:], 0.0)
            ue3 = ue[:].rearrange("p (s d) -> p s d", d=D)
            pe3 = pe[:].rearrange("p (s d) -> p s d", d=D)
            ne3 = ne[:].rearrange("p (s d) -> p s d", d=D)

            def masked_gather_f32(acc3, table_view, idx_d_, mask_d_):
                gtile = bsp.tile([128, NL * D], f32, tag="bg")
                itile = bsp.tile([128, BPC // 16], i16, tag="bidx")
                mtile = bsp.tile([128, NL], f32, tag="bmask")
                nc.sync.dma_start(out=itile[:], in_=idx_d_[:])
                nc.sync.dma_start(out=mtile[:], in_=mask_d_[:])
                nc.gpsimd.dma_gather(
                    out_ap=gtile[:].rearrange("p (s e) -> p s e", e=D),
                    in_ap=table_view,
                    idxs_ap=itile[:], num_idxs=BPC, num_idxs_reg=BPC,
                    elem_size=D, single_packet=False, queue_num=next_q())
                g3 = gtile[:].rearrange("p (s e) -> p s e", e=D)
                tmp = bsp.tile([128, NL * D], f32, tag="btmp")
                tmp3 = tmp[:].rearrange("p (s e) -> p s e", e=D)
                nc.vector.tensor_tensor(
                    out=tmp3, in0=g3,
                    in1=mtile[:, :, None].to_broadcast([128, NL, D]),
                    op=OP.mult)
                nc.vector.tensor_tensor(out=acc3, in0=acc3, in1=tmp3, op=OP.add)

            for r in range(NUB):
                masked_gather_f32(ue3, uemb_d[r * UBSZ:(r + 1) * UBSZ, :],
                                  uidx_d[r], umask_d[r])
            for r in range(NIB):
                sz = min(IBSZ, I - r * IBSZ)
                masked_gather_f32(pe3, iemb_d[r * IBSZ:r * IBSZ + sz, :],
                                  pidx_d[r], pmask_d[r])
                masked_gather_f32(ne3, iemb_d[r * IBSZ:r * IBSZ + sz, :],
                                  nidx_d[r], nmask_d[r])

            # scores + bce + reg1 (independent of softmax table)
            def rowdot(in0_3, in1_3, width, tag, odt=f32):
                prod = tp.tile([128, NL * width], odt, tag=f"{tag}_p")
                p3 = prod[:].rearrange("p (s e) -> p s e", e=width)
                nc.vector.tensor_tensor(out=p3, in0=in0_3, in1=in1_3, op=OP.mult)
                out = tp.tile([128, NL], f32, tag=f"{tag}_r")
                nc.vector.reduce_sum(out[:], p3, axis=AX.X)
                return out

            s_pos = rowdot(ue3, pe3, D, "sp")
            s_neg = rowdot(ue3, ne3, D, "sn")

            sigp = tp.tile([128, NL], f32, tag="sigp")
            nc.scalar.activation(sigp[:], s_pos[:], AF.Sigmoid)
            lsp = tp.tile([128, NL], f32, tag="lsp")
            nc.scalar.activation(lsp[:], sigp[:], AF.Ln)
            nc.vector.tensor_scalar(out=lsp[:], in0=lsp[:], scalar1=-100.0,
                                    scalar2=None, op0=OP.max)
            sign_ = tp.tile([128, NL], f32, tag="sign")
            nc.scalar.activation(sign_[:], s_neg[:], AF.Sigmoid, scale=-1.0)
            lsn = tp.tile([128, NL], f32, tag="lsn")
            nc.scalar.activation(lsn[:], sign_[:], AF.Ln)
            nc.vector.tensor_scalar(out=lsn[:], in0=lsn[:], scalar1=-100.0,
                                    scalar2=None, op0=OP.max)

            r1 = tp.tile([128, 1], f32, tag="r1")
            nc.vector.memset(r1[:], 0.0)
            for emb in (ue, pe, ne):
                sq = tp.tile([128, NL * D], f32, tag="r1sq")
                nc.vector.tensor_tensor(out=sq[:], in0=emb[:], in1=emb[:],
                                        op=OP.mult)
                rs = tp.tile([128, 1], f32, tag="r1rs")
                nc.vector.reduce_sum(rs[:], sq[:], axis=AX.X)
                nc.vector.tensor_tensor(out=r1[:], in0=r1[:], in1=rs[:], op=OP.add)
            nc.vector.tensor_copy(out=cols_t[:, 2:3], in_=r1[:])

            # ---------------- phase A: softmax -> padded bf16 DRAM table -------
            with tc.tile_pool(name="sm", bufs=2) as sm:
                for t in range(NTT + 1):
                    P = 128 if t < NTT else (U - NTT * 128 * QSM) // QSM
                    rows = slice(t * 128 * QSM, t * 128 * QSM + P * QSM)
                    tin = sm.tile([128, QSM * C], f32, tag="tin")
                    nc.sync.dma_start(
                        out=tin[:P],
                        in_=theta_d[rows, :].rearrange("(p q) c -> p (q c)", q=QSM))
                    te = sm.tile([128, QSM * C], f32, tag="te")
                    nc.scalar.activation(te[:P], tin[:P], AF.Exp)
                    te3 = te[:P].rearrange("p (q c) -> p q c", c=C)
                    ts = sm.tile([128, QSM], f32, tag="ts")
                    nc.vector.reduce_sum(ts[:P], te3, axis=AX.X)
                    tr = sm.tile([128, QSM], f32, tag="tr")
                    nc.vector.reciprocal(tr[:P], ts[:P])
                    tout = sm.tile([128, QSM * EPAD], bf16, tag="tout")
                    t3 = tout[:P].rearrange("p (q e) -> p q e", e=EPAD)
                    nc.gpsimd.memset(t3[:, :, C:], 0.0)
                    nc.vector.tensor_tensor(
                        out=t3[:, :, :C], in0=te3,
                        in1=tr[:P, :, None].to_broadcast([P, QSM, C]),
                        op=OP.mult)
                    nc.sync.dma_start(
                        out=tpad_d[rows, :].rearrange("(p q) e -> p (q e)", q=QSM),
                        in_=tout[:P])

            # ---- reg partials ----
            with tc.tile_pool(name="regs", bufs=1) as rp:
                thsq_t = rp.tile([128, (U // NCORES) * C // 128], f32, tag="thsq")
                nc.sync.dma_start(out=thsq_t[:], in_=thsq_d[:])
                thsq2 = rp.tile([128, (U // NCORES) * C // 128], f32, tag="thsq2")
                nc.vector.tensor_tensor(out=thsq2[:], in0=thsq_t[:], in1=thsq_t[:],
                                        op=OP.mult)
                nc.vector.reduce_sum(cols_t[:, 0:1], thsq2[:], axis=AX.X)
                wsq_t = rp.tile([128, 98], f32, tag="wsq")
                nc.sync.dma_start(out=wsq_t[:], in_=wsq_d[:])
                wsq2 = rp.tile([128, 98], f32, tag="wsq2")
                nc.vector.tensor_tensor(out=wsq2[:], in0=wsq_t[:], in1=wsq_t[:],
                                        op=OP.mult)
                nc.vector.reduce_sum(cols_t[:, 1:2], wsq2[:], axis=AX.X)

            bsp_ctx.__exit__(None, None, None)

            # ---------------- phase C: edges -> z -> z1pad ---------------------
            w1l_t = cpool.tile([128, NBLK], f32)
            w2l_t = cpool.tile([128, NBLK], f32)
            nc.sync.dma_start(out=w1l_t[:], in_=w1l_d[:])
            nc.sync.dma_start(out=w2l_t[:], in_=w2l_d[:])
            lo_full = cpool.tile([128, TOTCH], bf16)
            val_full = cpool.tile([128, TOTCH], bf16)
            nc.sync.dma_start(out=lo_full[:], in_=lo_d[:])
            nc.sync.dma_start(out=val_full[:], in_=val_d[:])
            iota_rep4 = cpool.tile([128, GRP * S * 128], bf16)
            nc.vector.tensor_copy(
                out=iota_rep4[:].rearrange("p (s q) -> p s q", q=128),
                in_=iota_t[:, None, :].to_broadcast([128, GRP * S, 128]))

            with tc.tile_pool(name="main", bufs=2) as mp, \
                 tc.tile_pool(name="mm", bufs=2) as mm, \
                 tc.tile_pool(name="mpool", bufs=1) as mq, \
                 tc.tile_pool(name="psum", bufs=2, space="PSUM") as pp:
                for g in range(NGRP):
                    gts = []
                    for r in range(NUB):
                        gi = g * NUB + r
                        gt = mp.tile([128, GRP * S * EPAD], bf16, tag=f"G{r}")
                        ci = mp.tile([128, NI // 16], i16, tag=f"ci{r}")
                        nc.sync.dma_start(
                            out=ci[:],
                            in_=cols16_d[:, gi * (NI // 16):(gi + 1) * (NI // 16)])
                        nc.gpsimd.dma_gather(
                            out_ap=gt[:].rearrange("p (s e) -> p s e", e=EPAD),
                            in_ap=tpad_d[r * UBSZ:(r + 1) * UBSZ, :],
                            idxs_ap=ci[:],
                            num_idxs=NI, num_idxs_reg=NI, elem_size=EPAD,
                            single_packet=False, queue_num=r)
                        gt3 = gt[:].rearrange("p (s e) -> p s e", e=EPAD)
                        cbase = gi * GRP * S
                        nc.vector.tensor_tensor(
                            out=gt3[:, :, :C], in0=gt3[:, :, :C],
                            in1=val_full[:, cbase:cbase + GRP * S, None]
                                .to_broadcast([128, GRP * S, C]),
                            op=OP.mult)
                        gts.append(gt)
                    mts = []
                    for r in range(NUB):
                        cbase = (g * NUB + r) * GRP * S
                        mt = mq.tile([128, GRP * S * 128], bf16, tag=f"M{r}")
                        nc.vector.tensor_tensor(
                            out=mt[:].rearrange("p (s q) -> p s q", q=128),
                            in0=iota_rep4[:].rearrange("p (s q) -> p s q", q=128),
                            in1=lo_full[:, cbase:cbase + GRP * S, None]
                                .to_broadcast([128, GRP * S, 128]),
                            op=OP.is_equal)
                        mts.append(mt)
                    for b_loc in range(GRP):
                        b = g * GRP + b_loc
                        zp = pp.tile([128, C], f32, space="PSUM", tag="zp")
                        for r in range(NUB):
                            mt = mts[r]
                            g3 = gts[r][:].rearrange("p (s e) -> p s e", e=EPAD)
                            for s in range(S):
                                nc.tensor.matmul(
                                    out=zp[:],
                                    lhsT=mt[:, (b_loc * S + s) * 128:
                                            (b_loc * S + s + 1) * 128],
                                    rhs=g3[:, b_loc * S + s, :C],
                                    start=(r == 0 and s == 0),
                                    stop=(r == NUB - 1 and s == S - 1))
                        z1s = mm.tile([128, EPAD], bf16, tag="z1s")
                        nc.vector.memset(z1s[:, C:], 0.0)
                        nc.scalar.activation(z1s[:, :C], zp[:], AF.Sigmoid,
                                             bias=w2l_t[:, b:b + 1],
                                             scale=w1l_t[:, b:b + 1])
                        nc.sync.dma_start(out=z1pad_d[b * 128:(b + 1) * 128, :],
                                          in_=z1s[:])

            # ---------------- phase D: softmax-dependent batch tail ------------
            thu = tp.tile([128, NL * C], bf16, tag="thu")
            nc.vector.memset(thu[:], 0.0)
            thu3 = thu[:].rearrange("p (s c) -> p s c", c=C)
            for r in range(NUB):
                gtile = tp.tile([128, NL * EPAD], bf16, tag="tg")
                itile = tp.tile([128, BPC // 16], i16, tag="tgidx")
                mtile = tp.tile([128, NL], bf16, tag="tgmask")
                mtile_f = tp.tile([128, NL], f32, tag="tgmaskf")
                nc.sync.dma_start(out=itile[:], in_=uidx_d[r][:])
                nc.sync.dma_start(out=mtile_f[:], in_=umask_d[r][:])
                nc.vector.tensor_copy(out=mtile[:], in_=mtile_f[:])
                nc.gpsimd.dma_gather(
                    out_ap=gtile[:].rearrange("p (s e) -> p s e", e=EPAD),
                    in_ap=tpad_d[r * UBSZ:(r + 1) * UBSZ, :],
                    idxs_ap=itile[:], num_idxs=BPC, num_idxs_reg=BPC,
                    elem_size=EPAD, single_packet=False, queue_num=next_q())
                g3 = gtile[:].rearrange("p (s e) -> p s e", e=EPAD)
                tmp = tp.tile([128, NL * C], bf16, tag="ttmp")
                tmp3 = tmp[:].rearrange("p (s e) -> p s e", e=C)
                nc.vector.tensor_tensor(
                    out=tmp3, in0=g3[:, :, :C],
                    in1=mtile[:, :, None].to_broadcast([128, NL, C]),
                    op=OP.mult)
                nc.vector.tensor_tensor(out=thu3, in0=thu3, in1=tmp3, op=OP.add)

            def plain_gather(slot_d_, tag):
                gtile = tp.tile([128, NL * EPAD], bf16, tag=f"zg{tag}")
                itile = tp.tile([128, BPC // 16], i16, tag=f"zi{tag}")
                nc.sync.dma_start(out=itile[:], in_=slot_d_[:])
                nc.gpsimd.dma_gather(
                    out_ap=gtile[:].rearrange("p (s e) -> p s e", e=EPAD),
                    in_ap=z1pad_d[:],
                    idxs_ap=itile[:], num_idxs=BPC, num_idxs_reg=BPC,
                    elem_size=EPAD, single_packet=False, queue_num=next_q())
                return gtile[:].rearrange("p (s e) -> p s e", e=EPAD)

            z1p3 = plain_gather(pslot_d, "p")
            z1n3 = plain_gather(nslot_d, "n")

            g_pos = rowdot(thu3, z1p3[:, :, :C], C, "gp", odt=f32)
            g_neg = rowdot(thu3, z1n3[:, :, :C], C, "gn", odt=f32)

            mf1 = tp.tile([128, NL], f32, tag="mf1")
            nc.vector.tensor_tensor(out=mf1[:], in0=g_pos[:], in1=lsp[:],
                                    op=OP.mult)
            mf2 = tp.tile([128, NL], f32, tag="mf2")
            nc.vector.tensor_tensor(out=mf2[:], in0=g_neg[:], in1=lsn[:],
                                    op=OP.mult)
            nc.vector.tensor_tensor(out=mf1[:], in0=mf1[:], in1=mf2[:], op=OP.add)
            nc.vector.tensor_scalar(out=mf1[:], in0=mf1[:], scalar1=-1.0,
                                    scalar2=None, op0=OP.mult)
            nc.vector.reduce_sum(cols_t[:, 3:4], mf1[:], axis=AX.X)

            nc.vector.reduce_sum(cols_t[:, 4:5], g_pos[:], axis=AX.X)
            nc.vector.reduce_sum(cols_t[:, 5:6], g_neg[:], axis=AX.X)

            gu_acc = tp.tile([128, NL], f32, tag="gu")
            nc.vector.memset(gu_acc[:], 0.0)
            for gg in (g_pos, g_neg):
                lg = tp.tile([128, NL], f32, tag="lg")
                nc.scalar.activation(lg[:], gg[:], AF.Ln)
                nc.vector.tensor_scalar(out=lg[:], in0=lg[:], scalar1=-100.0,
                                        scalar2=None, op0=OP.max)
                omg = tp.tile([128, NL], f32, tag="omg")
                nc.vector.tensor_scalar(out=omg[:], in0=gg[:], scalar1=-1.0,
                                        scalar2=1.0, op0=OP.mult, op1=OP.add)
                l1g = tp.tile([128, NL], f32, tag="l1g")
                nc.scalar.activation(l1g[:], omg[:], AF.Ln)
                nc.vector.tensor_scalar(out=l1g[:], in0=l1g[:], scalar1=-100.0,
                                        scalar2=None, op0=OP.max)
                t1 = tp.tile([128, NL], f32, tag="gu_t1")
                nc.vector.tensor_tensor(out=t1[:], in0=gg[:], in1=lg[:], op=OP.mult)
                t2 = tp.tile([128, NL], f32, tag="gu_t2")
                nc.vector.tensor_tensor(out=t2[:], in0=omg[:], in1=l1g[:],
                                        op=OP.mult)
                nc.vector.tensor_tensor(out=t1[:], in0=t1[:], in1=t2[:], op=OP.add)
                nc.vector.tensor_tensor(out=gu_acc[:], in0=gu_acc[:], in1=t1[:],
                                        op=OP.add)
            nc.vector.tensor_scalar(out=gu_acc[:], in0=gu_acc[:], scalar1=-1.0,
                                    scalar2=None, op0=OP.mult)
            nc.vector.reduce_sum(cols_t[:, 6:7], gu_acc[:], axis=AX.X)

            with tc.tile_pool(name="fps", bufs=1, space="PSUM") as fp:
                pout = fp.tile([1, 16], f32, space="PSUM")
                nc.tensor.matmul(out=pout[:], lhsT=ones_t[:], rhs=cols_t[:],
                                 start=True, stop=True)
                pres = tp.tile([1, 16], f32, tag="pres")
                nc.vector.tensor_copy(out=pres[:], in_=pout[:])
                nc.sync.dma_start(out=parts_d[:], in_=pres[:])

    nc.compile()

    # Align each gather's SWDGE queue with the DMASW semaphore lane Tile
    # assigned it (lane k -> queue k % NQ). Tile's round-robin lane assignment
    # ignores queue_num, and ucode locks each completion sem to one queue.
    import re
    for f in nc.m.functions:
        for blk in f.blocks:
            for inst in blk.instructions:
                if isinstance(inst, mybir.InstDMAGatherAnt):
                    for up in inst.sync_info.on_update:
                        m_ = re.match(r"DMASW(\d+)_", up.ant_name or "")
                        if m_:
                            inst.queue_num = int(m_.group(1)) % NQ

    _BUILD_CACHE[S] = nc
    return nc


def run(inputs, trace=False, trace_kwargs=None):
    from concourse.bass_utils import run_bass_kernel_spmd
    in_maps, S = _prep(inputs)
    nc = _build(S)
    kw = {}
    if trace:
        kw["trace"] = True
        if trace_kwargs:
            kw.update(trace_kwargs)
    res = run_bass_kernel_spmd(nc, in_maps, core_ids=list(range(NCORES)), **kw)
    parts = [res.results[k]["parts"] for k in range(NCORES)]
    loss = _combine(parts)
    return loss, res


def kernel(**inputs) -> np.ndarray:
    loss, _ = run(inputs)
    return np.asarray(loss, dtype=np.float32)
